# revision 27
# baseline (speedup 1.0000x reference)
"""Trainium2 Bass kernel for nn_DecoderAttention (sparse kNN attention block).

Sharding: core c handles batch n = c//2, parity p = c%2, owning q-tiles
{p, p+2, p+4, p+6} of the sequence (parity-interleaved for causal load
balance). No collectives: each core computes its 512 output rows end-to-end.

Top-128-of-row selection: 3 Newton + 4 Illinois count-bisection iterations
(fused is_ge+accum DVE passes) + exact max8 finisher that picks the
(count(lo)-k)-th smallest kept value as the threshold (tie-immune).
"""
import sys, math
from contextlib import ExitStack

sys.path.insert(0, "/opt/trn_rl_repo")

import numpy as np
import concourse.bass as bass
from concourse.bacc import Bacc
import concourse.mybir as mybir
from concourse.tile import TileContext
from concourse.bass import ts, ds

F32 = mybir.dt.float32
BF16 = mybir.dt.bfloat16
AF = mybir.ActivationFunctionType
ALU = mybir.AluOpType
AXX = mybir.AxisListType.X

H, KD, VD, KNN = 8, 64, 64, 128
D, FC, N, S = 512, 2048, 4, 1024
SCALE = 8.0
EPS = 1e-5
EB = 4.0            # e = exp(att_raw/(SCALE) - EB)
NEGBIG = -1.0e18    # causal additive mask
DROP = -2.0e21      # finisher drop penalty
B_NEWTON = 3
B_ILL = 4
SLACK = 6.0
NSLOT = 4
W_SLOT = [256, 512, 768, 1024]  # layer-1 active widths per slot (covers both parities)


def _inv_norm(p):
    lo, hi = -8.0, 8.0
    for _ in range(80):
        m = 0.5 * (lo + hi)
        if 0.5 * (1 + math.erf(m / math.sqrt(2))) < p:
            lo = m
        else:
            hi = m
    return 0.5 * (lo + hi)


def _sel_tables(widths):
    w = np.asarray(widths, np.float64)
    k = np.minimum(w, float(KNN))
    pq = np.clip(1.0 - k / w, 1e-4, 1.0 - 1e-6)
    z0 = np.array([_inv_norm(v) for v in pq])
    phi = np.exp(-z0 * z0 / 2) / math.sqrt(2 * math.pi)
    c0 = np.clip(1.0 / (w * phi), 0.0, 1.0)
    flo0 = w - (k - 0.5)
    km = k - 0.5
    return np.stack([z0, c0, flo0, km], -1).astype(np.float32)  # [128, 4]


# ---------------------------------------------------------------------------
def build_program():
    nc = bass.Bass()

    def din(name, shape, dtype=F32):
        return nc.dram_tensor(name, shape, dtype, kind="ExternalInput")

    yT = din("yT", (128, 4, S))
    yTq = din("yTq", (128, 4, 512))
    zT = din("zT", (128, 4, S))
    y_eff = din("y_eff", (128, NSLOT * D))
    gdec = din("gdec", (NSLOT, 128, S))
    genc = din("genc", (NSLOT, 128, S))
    wk_dec = din("wk_dec", (128, 4, 512))
    wv_dec = din("wv_dec", (128, 4, 512))
    wk_enc = din("wk_enc", (128, 4, 512))
    wq_enc = din("wq_enc", (128, 4, 512))
    wv_enc = din("wv_enc", (128, 4, 512))
    bk_dec = din("bk_dec", (64, H))
    bk_enc = din("bk_enc", (64, H))
    bq_enc = din("bq_enc", (64, H))
    wo_dec = din("wo_dec", (128, 4, 512), BF16)
    wo_enc = din("wo_enc", (128, 4, 512), BF16)
    bo_enc_b = din("bo_enc_b", (128, D))
    w1 = din("w1", (128, 4, FC), BF16)
    w2 = din("w2", (128, 16, 512), BF16)
    b1T = din("b1T", (128, FC // 128))
    b2_b = din("b2_b", (128, D))
    cmask = din("cmask", (NSLOT, 128, S))
    seltab = din("seltab", (128, 2 * NSLOT * 4))
    rsel = din("rsel", (8, 512))
    iota8 = din("iota8", (128, 8))
    ident_in = din("ident", (128, 128))
    out = nc.dram_tensor("out", (NSLOT, 128, D), F32, kind="ExternalOutput")
    dbg_h1 = nc.dram_tensor("dbg_h1", (128, NSLOT * D), F32, kind="ExternalOutput")
    dbg_kt = nc.dram_tensor("dbg_kt", (128, 4, 128), F32, kind="ExternalOutput")
    dbg_ts = nc.dram_tensor("dbg_ts", (NSLOT, 128, 8), F32, kind="ExternalOutput")
    dbg_sel = nc.dram_tensor("dbg_sel", (NSLOT, 128, 32), F32, kind="ExternalOutput")

    with TileContext(nc) as tc, ExitStack() as ectx:
        cp = ectx.enter_context(tc.tile_pool(name="const", bufs=1))
        wp = ectx.enter_context(tc.tile_pool(name="work", bufs=2))
        scp = ectx.enter_context(tc.tile_pool(name="scr", bufs=3))
        sp = ectx.enter_context(tc.tile_pool(name="state", bufs=1))
        pp = ectx.enter_context(tc.tile_pool(name="psum", bufs=2, space="PSUM"))
        pc = ectx.enter_context(tc.tile_pool(name="psumctx", bufs=1, space="PSUM"))

        def ps512():
            return pp.tile([128, 512], F32, tag="ps512", name="ps512")

        def load(ap_dram, shape, dtype=F32, pool=cp, name=None, funnel=True):
            t = pool.tile(shape, dtype, tag=name, name=name)
            nc.gpsimd.dma_start(t[:], ap_dram)
            if funnel:
                # collapse the multi-queue DMA into a single-producer so
                # LDWEIGHTS consumers only need one sync wait
                nc.scalar.copy(t[:], t[:])
            return t

        yT_sb = load(yT[:, :, :], [128, 4, S], name="yT")
        yTq_sb = load(yTq[:, :, :], [128, 4, 512], name="yTq")
        
        wkd_sb = load(wk_dec[:, :, :], [128, 4, 512], name="wkd", funnel=True)
        wvd_sb = load(wv_dec[:, :, :], [128, 4, 512], name="wvd", funnel=True)
        
        
        
        bkd_sb = load(bk_dec[:, :], [64, H], name="bkd")
        bke_sb = load(bk_enc[:, :], [64, H], name="bke")
        bqe_sb = load(bq_enc[:, :], [64, H], name="bqe")
        wod_sb = load(wo_dec[:, :, :], [128, 4, 512], BF16, name="wod", funnel=True)
        
        boe_sb = load(bo_enc_b[:, :], [128, D], name="boe")
        b1_sb = load(b1T[:, :], [128, FC // 128], name="b1")
        b2_sb = load(b2_b[:, :], [128, D], name="b2")
        selt_sb = load(seltab[:, :], [128, 2 * NSLOT * 4], name="selt")
        rsel_sb = load(rsel[:, :], [8, 512], name="rsel", funnel=True)
        iota_sb = load(iota8[:, :], [128, 8], name="iota8")
        yeff_sb = load(y_eff[:, :], [128, NSLOT * D], name="yeff")
        ident_sb = load(ident_in[:, :], [128, 128], name="ident", funnel=True)

        ones8 = cp.tile([128, 8], F32, tag="ones8")
        nc.vector.memset(ones8[:], 1.0)
        ones1 = cp.tile([128, 1], F32, tag="ones1")
        nc.vector.memset(ones1[:], 1.0)
        drop1 = cp.tile([128, 1], F32, tag="drop1")
        nc.vector.memset(drop1[:], DROP)
        cNEB = cp.tile([128, 1], F32, tag="cNEB")
        nc.vector.memset(cNEB[:], -EB)
        cEPS = cp.tile([128, 1], F32, tag="cEPS")
        nc.vector.memset(cEPS[:], EPS)

        def selt(layer, slot, col):
            c = ((layer * NSLOT) + slot) * 4 + col
            return selt_sb[:, c:c + 1]

        # ------------------------------------------------------------------
        def softmax_half_T(g_dram_slot, out_gT):
            g = scp.tile([128, S], F32, tag="scr1024", name="g")
            nc.gpsimd.dma_start(g[:], g_dram_slot)
            mx = wp.tile([128, 1], F32, tag="gmx")
            nc.vector.tensor_reduce(mx[:], g[:], op=ALU.max, axis=AXX)
            nmx = wp.tile([128, 1], F32, tag="gnmx")
            nc.vector.tensor_scalar(out=nmx[:], in0=mx[:], scalar1=-1.0, scalar2=None,
                                    op0=ALU.mult)
            e = scp.tile([128, S], F32, tag="scr1024", name="gse")
            ssum = wp.tile([128, 1], F32, tag="gsum")
            nc.scalar.activation(out=e[:], in_=g[:], func=AF.Exp, bias=nmx[:], scale=1.0,
                                 accum_out=ssum[:])
            rec = wp.tile([128, 1], F32, tag="grec")
            nc.vector.reciprocal(out=rec[:], in_=ssum[:])
            nc.vector.tensor_scalar(out=rec[:], in0=rec[:], scalar1=0.5, scalar2=None,
                                    op0=ALU.mult)
            gb = wp.tile([128, S], BF16, tag="gbf")
            nc.vector.tensor_scalar(out=gb[:], in0=e[:], scalar1=rec[:], scalar2=None,
                                    op0=ALU.mult)
            for kt in range(8):
                nc.sync.dma_start_transpose(out_gT[:, kt, :], gb[:, ts(kt, 128)])

        def project_T(xT_sb, w_sb, b_sb, outT, width):
            """outT [128, 4, width] f32, head h at partitions (h%2)*64..+64 of pair h//2."""
            for h in range(H):
                pt, po = h // 2, (h % 2) * 64
                nmm = (width + 511) // 512
                for m in range(nmm):
                    wfree = min(512, width - m * 512)
                    ps = ps512()
                    for dt_ in range(4):
                        nc.tensor.matmul(ps[:64, :wfree],
                                         lhsT=w_sb[:, dt_, ds(h * 64, 64)],
                                         rhs=xT_sb[:, dt_, ds(m * 512, wfree)],
                                         start=(dt_ == 0), stop=(dt_ == 3))
                    nc.scalar.activation(out=outT[ds(po, 64), pt, ds(m * 512, wfree)],
                                         in_=ps[:64, :wfree], func=AF.Identity,
                                         bias=b_sb[:, h:h + 1], scale=1.0)

        def project_V(xT_sb, w_sb, outV):
            """outV [128, 8, 512] bf16 = x @ Wv (no bias), k-tile major."""
            for kt in range(8):
                ps = ps512()
                for dt_ in range(4):
                    nc.tensor.matmul(ps[:], lhsT=xT_sb[:, dt_, ts(kt, 128)],
                                     rhs=w_sb[:, dt_, :],
                                     start=(dt_ == 0), stop=(dt_ == 3))
                nc.scalar.activation(out=outV[:, kt, :], in_=ps[:], func=AF.Copy, scale=1.0)

        def _layernorm(x_sb, out_ap):
            st = wp.tile([128, 1, 6], F32, tag="lnst")
            nc.vector.bn_stats(out=st[:], in_=x_sb[:, :])
            ag = wp.tile([128, 2], F32, tag="lnag")
            nc.vector.bn_aggr(out=ag[:], in_=st[:])
            sdv = wp.tile([128, 1], F32, tag="lnsd")
            nc.scalar.activation(out=sdv[:], in_=ag[:, 1:2], func=AF.Sqrt, bias=cEPS[:], scale=1.0)
            nc.vector.reciprocal(out=sdv[:], in_=sdv[:])
            nc.vector.tensor_scalar(out=out_ap, in0=x_sb[:], scalar1=ag[:, 0:1],
                                    scalar2=sdv[:], op0=ALU.subtract, op1=ALU.mult)

        # ------------------------------------------------------------------
        def attention_layer(layer, KT_sb, V_sb, QT_sb, gT_all, h_out, resid_fn, wo_sb):
            for j in range(NSLOT):
                Wj = W_SLOT[j] if layer == 0 else S
                nkt = Wj // 128
                nch = Wj // 256
                att = sp.tile([128, 8, S], F32, tag="att")
                if layer == 0:
                    msk = sp.tile([128, S], F32, tag="cmaskt")
                    nc.gpsimd.dma_start(msk[:, :Wj], cmask[j, :, :Wj])
                t_ = sp.tile([128, 8], F32, tag="t_")
                lo = sp.tile([128, 8], F32, tag="lo")
                hi = sp.tile([128, 8], F32, tag="hi")
                SL = sp.tile([128, 2, 8], F32, tag="SL")    # [FLO, WLO]
                SH = sp.tile([128, 2, 8], F32, tag="SH")
                newv = sp.tile([128, 2, 8], F32, tag="newv")  # [f, ones]
                cnt = sp.tile([128, 8], F32, tag="cnt")
                f = newv[:, 0, :]
                sdc0 = sp.tile([128, 8], F32, tag="sdc0")
                mv = sp.tile([128, 8, 2], F32, tag="mv")
                zrec = sp.tile([128, 8], F32, tag="zrec")
                sd = sp.tile([128, 8], F32, tag="sd")
                ge = sp.tile([128, 8], mybir.dt.uint8, tag="ge")
                nge = sp.tile([128, 8], mybir.dt.uint8, tag="nge")
                stp = sp.tile([128, 8], F32, tag="stp")

                # ---- att matmuls + stats + causal mask ----
                for h in range(H):
                    ps = pp.tile([128, S], F32, tag="ps1024")
                    nmm = (Wj + 511) // 512
                    pt, po = h // 2, (h % 2) * 64
                    for m in range(nmm):
                        wfree = min(512, Wj - m * 512)
                        nc.tensor.matmul(ps[:, ds(m * 512, wfree)],
                                         lhsT=QT_sb[ds(po, 64), pt, ds(j * 128, 128)],
                                         rhs=KT_sb[ds(po, 64), pt, ds(m * 512, wfree)],
                                         start=True, stop=True,
                                         tile_position=(po, 0))
                    bnst = wp.tile([128, 2, 6], F32, tag="bnst")
                    nbc = (Wj + 511) // 512
                    for cch in range(nbc):
                        cw = min(512, Wj - cch * 512)
                        nc.vector.bn_stats(out=bnst[:, cch, :],
                                           in_=ps[:, ds(cch * 512, cw)])
                    nc.vector.bn_aggr(out=mv[:, h, :], in_=bnst[:, :nbc, :])
                    if layer == 0:
                        nc.vector.tensor_tensor(out=att[:, h, :Wj], in0=ps[:, :Wj],
                                                in1=msk[:, :Wj], op=ALU.add)
                    else:
                        nc.scalar.activation(out=att[:, h, :Wj], in_=ps[:, :Wj],
                                             func=AF.Copy, scale=1.0)

                # ---- selection init ----
                nc.scalar.activation(out=sd[:], in_=mv[:, :, 1], func=AF.Sqrt, scale=1.0)
                nc.vector.tensor_scalar(out=sdc0[:], in0=sd[:], scalar1=selt(layer, j, 1),
                                        scalar2=None, op0=ALU.mult)
                nc.vector.tensor_scalar(out=t_[:], in0=sd[:], scalar1=selt(layer, j, 0),
                                        scalar2=None, op0=ALU.mult)
                nc.vector.tensor_tensor(out=t_[:], in0=t_[:], in1=mv[:, :, 0], op=ALU.add)
                nc.vector.tensor_scalar(out=lo[:], in0=sd[:], scalar1=-8.0, scalar2=None,
                                        op0=ALU.mult)
                nc.vector.tensor_tensor(out=lo[:], in0=lo[:], in1=mv[:, :, 0], op=ALU.add)
                nc.vector.tensor_scalar(out=hi[:], in0=sd[:], scalar1=8.0, scalar2=None,
                                        op0=ALU.mult)
                nc.vector.tensor_tensor(out=hi[:], in0=hi[:], in1=mv[:, :, 0], op=ALU.add)
                nc.vector.tensor_scalar(out=SL[:, 0, :], in0=ones8[:],
                                        scalar1=selt(layer, j, 2), scalar2=None, op0=ALU.mult)
                nc.vector.memset(SL[:, 1, :], 1.0)
                nc.vector.tensor_scalar(out=SH[:, 0, :], in0=ones8[:],
                                        scalar1=selt(layer, j, 3), scalar2=-1.0,
                                        op0=ALU.mult, op1=ALU.mult)
                nc.vector.memset(SH[:, 1, :], 1.0)
                nc.vector.memset(newv[:, 1, :], 1.0)

                # ---- iterations ----
                for it in range(B_NEWTON + B_ILL):
                    for h in range(H):
                        junk = scp.tile([128, S], F32, tag="scr1024", name="junk")
                        nc.vector.scalar_tensor_tensor(out=junk[:, :Wj], in0=att[:, h, :Wj],
                                                       scalar=t_[:, h:h + 1],
                                                       in1=ones1[:].to_broadcast([128, Wj]),
                                                       op0=ALU.is_ge, op1=ALU.mult,
                                                       accum_out=cnt[:, h:h + 1])
                    nc.vector.tensor_scalar(out=f, in0=cnt[:], scalar1=selt(layer, j, 3),
                                            scalar2=None, op0=ALU.subtract)
                    nc.vector.tensor_scalar(out=ge[:], in0=f, scalar1=0.0, scalar2=None,
                                            op0=ALU.is_ge)
                    nc.vector.tensor_scalar(out=nge[:], in0=f, scalar1=0.0, scalar2=None,
                                            op0=ALU.is_lt)
                    nc.vector.tensor_scalar(out=SL[:, 1, :], in0=SL[:, 1, :], scalar1=0.5,
                                            scalar2=None, op0=ALU.mult)
                    nc.vector.tensor_scalar(out=SH[:, 1, :], in0=SH[:, 1, :], scalar1=0.5,
                                            scalar2=None, op0=ALU.mult)
                    nc.vector.copy_predicated(lo[:], ge[:], t_[:])
                    nc.vector.copy_predicated(hi[:], nge[:], t_[:])
                    nc.vector.copy_predicated(
                        SL[:, :, :], ge[:, None, :].to_broadcast([128, 2, 8]), newv[:, :, :])
                    nc.vector.copy_predicated(
                        SH[:, :, :], nge[:, None, :].to_broadcast([128, 2, 8]), newv[:, :, :])
                    if it < B_NEWTON:
                        tgt = [0.0, SLACK, -SLACK][it]
                        nc.vector.tensor_scalar(out=stp[:], in0=f, scalar1=tgt, scalar2=None,
                                                op0=ALU.subtract)
                        nc.vector.tensor_tensor(out=stp[:], in0=stp[:], in1=sdc0[:],
                                                op=ALU.mult)
                        nc.vector.tensor_tensor(out=t_[:], in0=t_[:], in1=stp[:], op=ALU.add)
                        nc.vector.tensor_tensor(out=t_[:], in0=t_[:], in1=lo[:], op=ALU.max)
                        nc.vector.tensor_tensor(out=t_[:], in0=t_[:], in1=hi[:], op=ALU.min)
                    else:
                        fl = sp.tile([128, 8], F32, tag="fl")
                        fh = sp.tile([128, 8], F32, tag="fh")
                        den = sp.tile([128, 8], F32, tag="den")
                        num = sp.tile([128, 8], F32, tag="num")
                        nc.vector.tensor_tensor(out=fl[:], in0=SL[:, 0, :], in1=SL[:, 1, :],
                                                op=ALU.mult)
                        nc.vector.tensor_tensor(out=fh[:], in0=SH[:, 0, :], in1=SH[:, 1, :],
                                                op=ALU.mult)
                        nc.vector.tensor_tensor(out=den[:], in0=fh[:], in1=fl[:],
                                                op=ALU.subtract)
                        nc.vector.reciprocal(out=den[:], in_=den[:])
                        nc.vector.tensor_tensor(out=num[:], in0=lo[:], in1=fh[:], op=ALU.mult)
                        nc.vector.tensor_tensor(out=stp[:], in0=hi[:], in1=fl[:], op=ALU.mult)
                        nc.vector.tensor_tensor(out=num[:], in0=num[:], in1=stp[:],
                                                op=ALU.subtract)
                        nc.vector.tensor_tensor(out=t_[:], in0=num[:], in1=den[:],
                                                op=ALU.mult)

                # ---- finisher ----
                idx = sp.tile([128, 8], F32, tag="idx")
                nc.vector.tensor_scalar(out=idx[:], in0=SL[:, 0, :], scalar1=0.5,
                                        scalar2=0.0, op0=ALU.subtract, op1=ALU.max)
                nc.vector.tensor_scalar(out=idx[:], in0=idx[:], scalar1=7.0, scalar2=None,
                                        op0=ALU.min)
                tstar = sp.tile([128, 8], F32, tag="tstar")
                for h in range(H):
                    wd = scp.tile([128, S], F32, tag="scr1024", name="wd")
                    nc.vector.scalar_tensor_tensor(out=wd[:, :Wj], in0=att[:, h, :Wj],
                                                   scalar=lo[:, h:h + 1],
                                                   in1=drop1[:].to_broadcast([128, Wj]),
                                                   op0=ALU.is_lt, op1=ALU.mult)
                    u = scp.tile([128, S], F32, tag="scr1024", name="u")
                    nc.vector.scalar_tensor_tensor(out=u[:, :Wj], in0=att[:, h, :Wj],
                                                   scalar=-1.0, in1=wd[:, :Wj],
                                                   op0=ALU.mult, op1=ALU.add)
                    u8 = wp.tile([128, 8], F32, tag="u8")
                    nc.vector.max(out=u8[:], in_=u[:, :Wj])
                    sel8 = wp.tile([128, 8], F32, tag="sel8")
                    nc.vector.tensor_tensor(out=sel8[:], in0=iota_sb[:],
                                            in1=idx[:, h:h + 1].to_broadcast([128, 8]),
                                            op=ALU.is_equal)
                    nc.vector.tensor_tensor(out=sel8[:], in0=sel8[:], in1=u8[:], op=ALU.mult)
                    nc.vector.tensor_reduce(tstar[:, h:h + 1], sel8[:], op=ALU.add, axis=AXX)
                nc.vector.tensor_scalar(out=tstar[:], in0=tstar[:], scalar1=-1.0,
                                        scalar2=None, op0=ALU.mult)
                if layer == 0:
                    nc.sync.dma_start(dbg_ts[j, :, :], tstar[:])
                    nc.sync.dma_start(dbg_sel[j, :, 0:8], lo[:])
                    nc.sync.dma_start(dbg_sel[j, :, 8:16], SL[:, 0, :])
                    nc.sync.dma_start(dbg_sel[j, :, 16:24], cnt[:])
                    nc.sync.dma_start(dbg_sel[j, :, 24:32], t_[:])

                # ---- exp, mask+Z, transpose, ctx ----
                psA = pc.tile([128, 512], F32, tag="psA")
                psB = pc.tile([128, 512], F32, tag="psB")
                for h in range(H):
                    e = scp.tile([128, S], F32, tag="scr1024", name="esb")
                    nc.scalar.activation(out=e[:, :Wj], in_=att[:, h, :Wj], func=AF.Exp,
                                         bias=cNEB[:], scale=1.0 / SCALE)
                    me = wp.tile([128, S], BF16, tag="mebf")
                    nc.vector.scalar_tensor_tensor(out=me[:, :Wj], in0=att[:, h, :Wj],
                                                   scalar=tstar[:, h:h + 1], in1=e[:, :Wj],
                                                   op0=ALU.is_ge, op1=ALU.mult,
                                                   accum_out=zrec[:, h:h + 1])
                    eT = wp.tile([128, 8, 128], BF16, tag="eT")
                    for kt in range(nkt):
                        nc.sync.dma_start_transpose(eT[:, kt, :], me[:, ts(kt, 128)])
                    tt, po = h // 2, (h % 2) * 64
                    for kt in range(nkt):
                        nc.tensor.matmul(psA[ds(po, 64), ts(tt, 128)],
                                         lhsT=V_sb[:, kt, ds(h * 64, 64)],
                                         rhs=eT[:, kt, :],
                                         start=(kt == 0), stop=(kt == nkt - 1),
                                         tile_position=(0, po))
                nc.vector.reciprocal(out=zrec[:], in_=zrec[:])
                for tt in range(4):
                    for kt in range(8):
                        nc.tensor.matmul(psB[:, ts(tt, 128)],
                                         lhsT=V_sb[:, kt, ts(tt, 128)],
                                         rhs=gT_all[:, j, kt, :],
                                         start=(kt == 0), stop=(kt == 7))
                zps = pp.tile([128, 512], F32, tag="ps512")
                nc.tensor.transpose(zps[:8, :128], zrec[:], ident_sb[:])
                zT_s = wp.tile([8, 128], F32, tag="zTs")
                nc.scalar.activation(out=zT_s[:], in_=zps[:8, :128], func=AF.Copy, scale=1.0)
                ctxT = wp.tile([128, 4, 128], BF16, tag="ctxT")
                for tt in range(4):
                    smat = pp.tile([128, 512], F32, tag="ps512")
                    nc.tensor.matmul(smat[:, :128], lhsT=rsel_sb[:, ts(tt, 128)], rhs=zT_s[:],
                                     start=True, stop=True)
                    smat_sb = wp.tile([128, 128], F32, tag="smatsb")
                    nc.scalar.activation(out=smat_sb[:], in_=smat[:, :128], func=AF.Copy,
                                         scale=1.0)
                    tmp = wp.tile([128, 128], F32, tag="ctmp")
                    nc.vector.tensor_tensor(out=tmp[:], in0=psA[:, ts(tt, 128)],
                                            in1=smat_sb[:], op=ALU.mult)
                    nc.vector.tensor_tensor(out=ctxT[:, tt, :], in0=tmp[:],
                                            in1=psB[:, ts(tt, 128)], op=ALU.add)
                hps = pp.tile([128, 512], F32, tag="ps512")
                for tt in range(4):
                    nc.tensor.matmul(hps[:], lhsT=ctxT[:, tt, :], rhs=wo_sb[:, tt, :],
                                     start=(tt == 0), stop=(tt == 3))
                pre = wp.tile([128, D], F32, tag="lnpre")
                nc.vector.tensor_tensor(out=pre[:], in0=hps[:], in1=resid_fn(j), op=ALU.add)
                _layernorm(pre, h_out[:, ds(j * D, D)])

        # ===== layer 1 =====
        gT_dec = cp.tile([128, NSLOT, 8, 128], BF16, tag="gT")
        for j in range(NSLOT):
            softmax_half_T(gdec[j, :, :], gT_dec[:, j, :, :])
        KT_dec = cp.tile([128, 4, S], F32, tag="KTd")
        project_T(yT_sb, wkd_sb, bkd_sb, KT_dec, S)
        QT_dec = cp.tile([128, 4, 512], F32, tag="QTd")
        project_T(yTq_sb, wkd_sb, bkd_sb, QT_dec, 512)
        V_dec = cp.tile([128, 8, 512], BF16, tag="Vd")
        project_V(yT_sb, wvd_sb, V_dec)
        h_l1 = cp.tile([128, NSLOT * D], F32, tag="h_l1")

        def resid_dec(j):
            return yeff_sb[:, ds(j * D, D)]

        attention_layer(0, KT_dec, V_dec, QT_dec, gT_dec, h_l1, resid_dec, wod_sb)

        nc.sync.dma_start(dbg_h1[:, :], h_l1[:])
        nc.sync.dma_start(dbg_kt[:, :, :], KT_dec[:, :, 256:384])
        # ===== layer 2 =====
        wke_sb = load(wk_enc[:, :, :], [128, 4, 512], name="wkd", funnel=True)
        wqe_sb = load(wq_enc[:, :, :], [128, 4, 512], name="yTq", funnel=True)
        wve_sb = load(wv_enc[:, :, :], [128, 4, 512], name="wvd", funnel=True)
        woe_sb = load(wo_enc[:, :, :], [128, 4, 512], BF16, name="wod", funnel=True)
        gT_enc = cp.tile([128, NSLOT, 8, 128], BF16, tag="gT")
        for j in range(NSLOT):
            softmax_half_T(genc[j, :, :], gT_enc[:, j, :, :])
        zT_sb = load(zT[:, :, :], [128, 4, S], name="yT")
        KT_enc = cp.tile([128, 4, S], F32, tag="KTd")
        project_T(zT_sb, wke_sb, bke_sb, KT_enc, S)
        V_enc = cp.tile([128, 8, 512], BF16, tag="Vd")
        project_V(zT_sb, wve_sb, V_enc)
        hT = cp.tile([128, 4, 512], F32, tag="hT")
        for j in range(NSLOT):
            for dt_ in range(4):
                ps = ps512()
                nc.tensor.transpose(ps[:, :128], h_l1[:, ds(j * D + dt_ * 128, 128)],
                                    ident_sb[:])
                nc.scalar.activation(out=hT[:, dt_, ds(j * 128, 128)], in_=ps[:, :128],
                                     func=AF.Copy, scale=1.0)
        QT_enc = cp.tile([128, 4, 512], F32, tag="QTd")
        project_T(hT, wqe_sb, bqe_sb, QT_enc, 512)
        h_l2 = cp.tile([128, NSLOT * D], F32, tag="h_l2")

        def resid_enc(j):
            r = wp.tile([128, D], F32, tag="rese")
            nc.vector.tensor_tensor(out=r[:], in0=h_l1[:, ds(j * D, D)], in1=boe_sb[:],
                                    op=ALU.add)
            return r[:]

        attention_layer(1, KT_enc, V_enc, QT_enc, gT_enc, h_l2, resid_enc, woe_sb)

        # ===== MLP =====
        w1_sb = sp.tile([128, 4, FC], BF16, tag="att", name="w1_sb")
        nc.gpsimd.dma_start(w1_sb[:], w1[:, :, :])
        nc.scalar.copy(w1_sb[:], w1_sb[:])
        w2_sb = cp.tile([128, 16, 512], BF16, tag="KTd", name="w2_sb")
        nc.gpsimd.dma_start(w2_sb[:], w2[:, :, :])
        nc.scalar.copy(w2_sb[:], w2_sb[:])
        for j in range(NSLOT):
            h2T = sp.tile([128, 4, 128], BF16, tag="h2T")
            for dt_ in range(4):
                ps = ps512()
                nc.tensor.transpose(ps[:, :128], h_l2[:, ds(j * D + dt_ * 128, 128)],
                                    ident_sb[:])
                nc.scalar.activation(out=h2T[:, dt_, :], in_=ps[:, :128], func=AF.Copy,
                                     scale=1.0)
            m1T = sp.tile([128, 16, 128], BF16, tag="cmaskt", name="m1T")
            for ft in range(16):
                ps = ps512()
                for dt_ in range(4):
                    nc.tensor.matmul(ps[:, :128], lhsT=w1_sb[:, dt_, ts(ft, 128)],
                                     rhs=h2T[:, dt_, :],
                                     start=(dt_ == 0), stop=(dt_ == 3))
                nc.scalar.activation(out=m1T[:, ft, :], in_=ps[:, :128], func=AF.Relu,
                                     bias=b1_sb[:, ft:ft + 1], scale=1.0)
            h3ps = pp.tile([128, 512], F32, tag="ps512")
            for ft in range(16):
                nc.tensor.matmul(h3ps[:], lhsT=m1T[:, ft, :], rhs=w2_sb[:, ft, :],
                                 start=(ft == 0), stop=(ft == 15))
            pre = wp.tile([128, D], F32, tag="mlppre")
            nc.vector.tensor_tensor(out=pre[:], in0=h3ps[:], in1=h_l2[:, ds(j * D, D)],
                                    op=ALU.add)
            nc.vector.tensor_tensor(out=pre[:], in0=pre[:], in1=b2_sb[:], op=ALU.add)
            o = wp.tile([128, D], F32, tag="osb")
            _layernorm(pre, o[:])
            nc.sync.dma_start(out[j, :, :], o[:])

    from concourse import bacc as _bacc
    _bacc._bass_rust.move_matmul_waits_to_ldweights(nc.m)
    _bacc._bass_rust.generate_event_semaphores(nc)
    return nc


# ---------------------------------------------------------------------------
# Host side
# ---------------------------------------------------------------------------

def _core_inputs(inputs, core):
    n, p = core // 2, core % 2
    G = [p + 2 * j for j in range(NSLOT)]          # global q-tile indices
    qrows = np.concatenate([np.arange(g * 128, g * 128 + 128) for g in G])

    y = np.asarray(inputs["y"], np.float32)[n]     # [S, D]
    z = np.asarray(inputs["z"], np.float32)[n]
    f32 = np.float32

    def P3(arr, a):
        arr = np.asarray(arr)
        return np.ascontiguousarray(arr.reshape(a, 128, arr.shape[-1]).transpose(1, 0, 2))

    def hmat(w):   # [H, D, KD] -> [128, 4, H*KD... pre-permuted [D,H*KD]]
        return P3(np.ascontiguousarray(np.moveaxis(np.asarray(w, f32), 0, 1)
                                       .reshape(D, H * KD)), 4)

    dec_bv_flat = np.asarray(inputs["dec_bv"], f32).reshape(H * VD)
    enc_bv_flat = np.asarray(inputs["enc_bv"], f32).reshape(H * VD)
    bo_dec_eff = np.asarray(inputs["dec_bo"], f32) + dec_bv_flat @ np.asarray(
        inputs["dec_Wo"], f32)
    bo_enc_eff = np.asarray(inputs["enc_bo"], f32) + enc_bv_flat @ np.asarray(
        inputs["enc_Wo"], f32)

    cm = np.zeros((NSLOT, 128, S), f32)
    seltabs = np.zeros((2, NSLOT, 128, 4), f32)
    for j, g in enumerate(G):
        qidx = np.arange(g * 128, g * 128 + 128)
        col = np.arange(S)[None, :]
        cm[j] = np.where(col <= qidx[:, None], 0.0, NEGBIG)
        seltabs[0, j] = _sel_tables(qidx + 1.0)
        seltabs[1, j] = _sel_tables(np.full(128, float(S)))

    rsel = np.zeros((8, 512), f32)
    for h in range(8):
        rsel[h, h * 64:(h + 1) * 64] = 0.5

    d = {
        "yT": P3(np.ascontiguousarray(y.T), 4),
        "yTq": P3(np.ascontiguousarray(y.T[:, qrows]), 4),
        "zT": P3(np.ascontiguousarray(z.T), 4),
        "y_eff": np.ascontiguousarray(np.moveaxis((y[qrows] + bo_dec_eff).reshape(NSLOT, 128, D), 1, 0).reshape(128, NSLOT * D)),
        "gdec": np.ascontiguousarray(np.asarray(inputs["graph_dec"], f32)[qrows]
                                     .reshape(NSLOT, 128, S)),
        "genc": np.ascontiguousarray(np.asarray(inputs["graph_enc"], f32)[qrows]
                                     .reshape(NSLOT, 128, S)),
        "wk_dec": hmat(inputs["dec_Wk"]),
        "wv_dec": hmat(inputs["dec_Wv"]),
        "wk_enc": hmat(inputs["enc_Wk"]),
        "wq_enc": hmat(inputs["enc_Wq"]),
        "wv_enc": hmat(inputs["enc_Wv"]),
        "bk_dec": np.ascontiguousarray(np.asarray(inputs["dec_bk"], f32).T),   # [64, H]
        "bk_enc": np.ascontiguousarray(np.asarray(inputs["enc_bk"], f32).T),
        "bq_enc": np.ascontiguousarray(np.asarray(inputs["enc_bq"], f32).T),
        "wo_dec": P3(np.asarray(inputs["dec_Wo"], f32), 4),
        "wo_enc": P3(np.asarray(inputs["enc_Wo"], f32), 4),
        "bo_enc_b": np.tile(bo_enc_eff[None, :], (128, 1)),
        "w1": P3(np.asarray(inputs["fc_W1"], f32), 4),
        "w2": P3(np.asarray(inputs["fc_W2"], f32), 16),
        "b1T": np.ascontiguousarray(np.asarray(inputs["fc_b1"], f32)
                                    .reshape(FC // 128, 128).T),
        "b2_b": np.tile(np.asarray(inputs["fc_b2"], f32)[None, :], (128, 1)),
        "cmask": cm,
        "seltab": np.ascontiguousarray(np.moveaxis(seltabs, 2, 0).reshape(128, 2 * NSLOT * 4)),
        "rsel": rsel,
        "iota8": np.tile(np.arange(8, dtype=f32)[None, :], (128, 1)),
        "ident": np.eye(128, dtype=f32),
    }
    import ml_dtypes
    for k in ("wo_dec", "wo_enc", "w1", "w2"):
        d[k] = d[k].astype(ml_dtypes.bfloat16)
    return d


_CACHE = {}


def kernel(**inputs):
    from concourse.bass_utils import run_bass_kernel_spmd

    if "nc" not in _CACHE:
        _CACHE["nc"] = build_program()
    nc = _CACHE["nc"]

    core_ids = list(range(8))
    in_maps = [_core_inputs(inputs, c) for c in core_ids]
    res = run_bass_kernel_spmd(nc, in_maps, core_ids)

    out = np.zeros((N, S, D), np.float32)
    for c in core_ids:
        n, p = c // 2, c % 2
        o = res.results[c]["out"]          # [NSLOT, 128, D]
        for j in range(NSLOT):
            g = p + 2 * j
            out[n, g * 128:(g + 1) * 128, :] = o[j]
    return out


# revision 28
# speedup vs baseline: 2987.6125x; 2987.6125x over previous
"""Trainium2 Bass kernel for nn_DecoderAttention (sparse kNN attention block).

Sharding: core c handles batch n = c//2, parity p = c%2, owning q-tiles
{p, p+2, p+4, p+6} of the sequence (parity-interleaved for causal load
balance). No collectives: each core computes its 512 output rows end-to-end.

Top-128-of-row selection: 3 Newton + 4 Illinois count-bisection iterations
(fused is_ge+accum DVE passes) + exact max8 finisher that picks the
(count(lo)-k)-th smallest kept value as the threshold (tie-immune).
"""
import sys, math
from contextlib import ExitStack

sys.path.insert(0, "/opt/trn_rl_repo")

import numpy as np
import concourse.bass as bass
from concourse.bacc import Bacc
import concourse.mybir as mybir
from concourse.tile import TileContext
from concourse.bass import ts, ds

F32 = mybir.dt.float32
BF16 = mybir.dt.bfloat16
AF = mybir.ActivationFunctionType
ALU = mybir.AluOpType
AXX = mybir.AxisListType.X

H, KD, VD, KNN = 8, 64, 64, 128
D, FC, N, S = 512, 2048, 4, 1024
SCALE = 8.0
EPS = 1e-5
EB = 4.0            # e = exp(att_raw/(SCALE) - EB)
NEGBIG = -1.0e18    # causal additive mask
DROP = -2.0e21      # finisher drop penalty
B_NEWTON = 3
B_ILL = 4
SLACK = 6.0
NSLOT = 4
W_SLOT = [256, 512, 768, 1024]  # layer-1 active widths per slot (covers both parities)


def _inv_norm(p):
    lo, hi = -8.0, 8.0
    for _ in range(80):
        m = 0.5 * (lo + hi)
        if 0.5 * (1 + math.erf(m / math.sqrt(2))) < p:
            lo = m
        else:
            hi = m
    return 0.5 * (lo + hi)


def _sel_tables(widths):
    w = np.asarray(widths, np.float64)
    k = np.minimum(w, float(KNN))
    pq = np.clip(1.0 - k / w, 1e-4, 1.0 - 1e-6)
    z0 = np.array([_inv_norm(v) for v in pq])
    phi = np.exp(-z0 * z0 / 2) / math.sqrt(2 * math.pi)
    c0 = np.clip(1.0 / (w * phi), 0.0, 1.0)
    flo0 = w - (k - 0.5)
    km = k - 0.5
    return np.stack([z0, c0, flo0, km], -1).astype(np.float32)  # [128, 4]


# ---------------------------------------------------------------------------
def build_program():
    nc = bass.Bass()

    def din(name, shape, dtype=F32):
        return nc.dram_tensor(name, shape, dtype, kind="ExternalInput")

    yT = din("yT", (128, 4, S))
    yTq = din("yTq", (128, 4, 512))
    zT = din("zT", (128, 4, S))
    y_eff = din("y_eff", (128, NSLOT * D))
    gdec = din("gdec", (NSLOT, 128, S))
    genc = din("genc", (NSLOT, 128, S))
    wk_dec = din("wk_dec", (128, 4, 512))
    wv_dec = din("wv_dec", (128, 4, 512))
    wk_enc = din("wk_enc", (128, 4, 512))
    wq_enc = din("wq_enc", (128, 4, 512))
    wv_enc = din("wv_enc", (128, 4, 512))
    bk_dec = din("bk_dec", (64, H))
    bk_enc = din("bk_enc", (64, H))
    bq_enc = din("bq_enc", (64, H))
    wo_dec = din("wo_dec", (128, 4, 512), BF16)
    wo_enc = din("wo_enc", (128, 4, 512), BF16)
    bo_enc_b = din("bo_enc_b", (128, D))
    w1 = din("w1", (128, 4, FC), BF16)
    w2 = din("w2", (128, 16, 512), BF16)
    b1T = din("b1T", (128, FC // 128))
    b2_b = din("b2_b", (128, D))
    cmask = din("cmask", (NSLOT, 128, S))
    seltab = din("seltab", (128, 2 * NSLOT * 4))
    rsel = din("rsel", (8, 512))
    iota8 = din("iota8", (128, 8))
    ident_in = din("ident", (128, 128))
    out = nc.dram_tensor("out", (NSLOT, 128, D), F32, kind="ExternalOutput")

    with TileContext(nc) as tc, ExitStack() as ectx:
        cp = ectx.enter_context(tc.tile_pool(name="const", bufs=1))
        wp = ectx.enter_context(tc.tile_pool(name="work", bufs=2))
        scp = ectx.enter_context(tc.tile_pool(name="scr", bufs=3))
        sp = ectx.enter_context(tc.tile_pool(name="state", bufs=1))
        pp = ectx.enter_context(tc.tile_pool(name="psum", bufs=2, space="PSUM"))
        pc = ectx.enter_context(tc.tile_pool(name="psumctx", bufs=1, space="PSUM"))

        def ps512():
            return pp.tile([128, 512], F32, tag="ps512", name="ps512")

        def load(ap_dram, shape, dtype=F32, pool=cp, name=None, funnel=True):
            t = pool.tile(shape, dtype, tag=name, name=name)
            nc.gpsimd.dma_start(t[:], ap_dram)
            if funnel:
                # collapse the multi-queue DMA into a single-producer so
                # LDWEIGHTS consumers only need one sync wait
                nc.scalar.copy(t[:], t[:])
            return t

        yT_sb = load(yT[:, :, :], [128, 4, S], name="yT")
        yTq_sb = load(yTq[:, :, :], [128, 4, 512], name="yTq")
        
        wkd_sb = load(wk_dec[:, :, :], [128, 4, 512], name="wkd", funnel=True)
        wvd_sb = load(wv_dec[:, :, :], [128, 4, 512], name="wvd", funnel=True)
        
        
        
        bkd_sb = load(bk_dec[:, :], [64, H], name="bkd")
        bke_sb = load(bk_enc[:, :], [64, H], name="bke")
        bqe_sb = load(bq_enc[:, :], [64, H], name="bqe")
        wod_sb = load(wo_dec[:, :, :], [128, 4, 512], BF16, name="wod", funnel=True)
        
        boe_sb = load(bo_enc_b[:, :], [128, D], name="boe")
        b1_sb = load(b1T[:, :], [128, FC // 128], name="b1")
        b2_sb = load(b2_b[:, :], [128, D], name="b2")
        selt_sb = load(seltab[:, :], [128, 2 * NSLOT * 4], name="selt")
        rsel_sb = load(rsel[:, :], [8, 512], name="rsel", funnel=True)
        iota_sb = load(iota8[:, :], [128, 8], name="iota8")
        yeff_sb = load(y_eff[:, :], [128, NSLOT * D], name="yeff")
        ident_sb = load(ident_in[:, :], [128, 128], name="ident", funnel=True)

        ones8 = cp.tile([128, 8], F32, tag="ones8")
        nc.vector.memset(ones8[:], 1.0)
        ones1 = cp.tile([128, 1], F32, tag="ones1")
        nc.vector.memset(ones1[:], 1.0)
        drop1 = cp.tile([128, 1], F32, tag="drop1")
        nc.vector.memset(drop1[:], DROP)
        cNEB = cp.tile([128, 1], F32, tag="cNEB")
        nc.vector.memset(cNEB[:], -EB)
        cEPS = cp.tile([128, 1], F32, tag="cEPS")
        nc.vector.memset(cEPS[:], EPS)

        def selt(layer, slot, col):
            c = ((layer * NSLOT) + slot) * 4 + col
            return selt_sb[:, c:c + 1]

        # ------------------------------------------------------------------
        def softmax_half_T(g_dram_slot, out_gT):
            g = scp.tile([128, S], F32, tag="scr1024", name="g")
            nc.gpsimd.dma_start(g[:], g_dram_slot)
            mx = wp.tile([128, 1], F32, tag="gmx")
            nc.vector.tensor_reduce(mx[:], g[:], op=ALU.max, axis=AXX)
            nmx = wp.tile([128, 1], F32, tag="gnmx")
            nc.vector.tensor_scalar(out=nmx[:], in0=mx[:], scalar1=-1.0, scalar2=None,
                                    op0=ALU.mult)
            e = scp.tile([128, S], F32, tag="scr1024", name="gse")
            ssum = wp.tile([128, 1], F32, tag="gsum")
            nc.scalar.activation(out=e[:], in_=g[:], func=AF.Exp, bias=nmx[:], scale=1.0,
                                 accum_out=ssum[:])
            rec = wp.tile([128, 1], F32, tag="grec")
            nc.vector.reciprocal(out=rec[:], in_=ssum[:])
            nc.vector.tensor_scalar(out=rec[:], in0=rec[:], scalar1=0.5, scalar2=None,
                                    op0=ALU.mult)
            gb = wp.tile([128, S], BF16, tag="gbf")
            nc.vector.tensor_scalar(out=gb[:], in0=e[:], scalar1=rec[:], scalar2=None,
                                    op0=ALU.mult)
            for kt in range(8):
                nc.sync.dma_start_transpose(out_gT[:, kt, :], gb[:, ts(kt, 128)])

        def project_T(xT_sb, w_sb, b_sb, outT, width):
            """outT [128, 4, width] f32, head h at partitions (h%2)*64..+64 of pair h//2."""
            for h in range(H):
                pt, po = h // 2, (h % 2) * 64
                nmm = (width + 511) // 512
                for m in range(nmm):
                    wfree = min(512, width - m * 512)
                    ps = ps512()
                    for dt_ in range(4):
                        nc.tensor.matmul(ps[:64, :wfree],
                                         lhsT=w_sb[:, dt_, ds(h * 64, 64)],
                                         rhs=xT_sb[:, dt_, ds(m * 512, wfree)],
                                         start=(dt_ == 0), stop=(dt_ == 3))
                    nc.scalar.activation(out=outT[ds(po, 64), pt, ds(m * 512, wfree)],
                                         in_=ps[:64, :wfree], func=AF.Identity,
                                         bias=b_sb[:, h:h + 1], scale=1.0)

        def project_V(xT_sb, w_sb, outV):
            """outV [128, 8, 512] bf16 = x @ Wv (no bias), k-tile major."""
            for kt in range(8):
                ps = ps512()
                for dt_ in range(4):
                    nc.tensor.matmul(ps[:], lhsT=xT_sb[:, dt_, ts(kt, 128)],
                                     rhs=w_sb[:, dt_, :],
                                     start=(dt_ == 0), stop=(dt_ == 3))
                nc.scalar.activation(out=outV[:, kt, :], in_=ps[:], func=AF.Copy, scale=1.0)

        def _layernorm(x_sb, out_ap):
            st = wp.tile([128, 1, 6], F32, tag="lnst")
            nc.vector.bn_stats(out=st[:], in_=x_sb[:, :])
            ag = wp.tile([128, 2], F32, tag="lnag")
            nc.vector.bn_aggr(out=ag[:], in_=st[:])
            sdv = wp.tile([128, 1], F32, tag="lnsd")
            nc.scalar.activation(out=sdv[:], in_=ag[:, 1:2], func=AF.Sqrt, bias=cEPS[:], scale=1.0)
            nc.vector.reciprocal(out=sdv[:], in_=sdv[:])
            nc.vector.tensor_scalar(out=out_ap, in0=x_sb[:], scalar1=ag[:, 0:1],
                                    scalar2=sdv[:], op0=ALU.subtract, op1=ALU.mult)

        # ------------------------------------------------------------------
        def attention_layer(layer, KT_sb, V_sb, QT_sb, gT_all, h_out, resid_fn, wo_sb):
            for j in range(NSLOT):
                Wj = W_SLOT[j] if layer == 0 else S
                nkt = Wj // 128
                nch = Wj // 256
                att = sp.tile([128, 8, S], F32, tag="att")
                if layer == 0:
                    msk = sp.tile([128, S], F32, tag="cmaskt")
                    nc.gpsimd.dma_start(msk[:, :Wj], cmask[j, :, :Wj])
                t_ = sp.tile([128, 8], F32, tag="t_")
                lo = sp.tile([128, 8], F32, tag="lo")
                hi = sp.tile([128, 8], F32, tag="hi")
                SL = sp.tile([128, 2, 8], F32, tag="SL")    # [FLO, WLO]
                SH = sp.tile([128, 2, 8], F32, tag="SH")
                newv = sp.tile([128, 2, 8], F32, tag="newv")  # [f, ones]
                cnt = sp.tile([128, 8], F32, tag="cnt")
                f = newv[:, 0, :]
                sdc0 = sp.tile([128, 8], F32, tag="sdc0")
                mv = sp.tile([128, 8, 2], F32, tag="mv")
                zrec = sp.tile([128, 8], F32, tag="zrec")
                sd = sp.tile([128, 8], F32, tag="sd")
                ge = sp.tile([128, 8], mybir.dt.uint8, tag="ge")
                nge = sp.tile([128, 8], mybir.dt.uint8, tag="nge")
                stp = sp.tile([128, 8], F32, tag="stp")

                # ---- att matmuls + stats + causal mask ----
                for h in range(H):
                    ps = pp.tile([128, S], F32, tag="ps1024")
                    nmm = (Wj + 511) // 512
                    pt, po = h // 2, (h % 2) * 64
                    for m in range(nmm):
                        wfree = min(512, Wj - m * 512)
                        nc.tensor.matmul(ps[:, ds(m * 512, wfree)],
                                         lhsT=QT_sb[ds(po, 64), pt, ds(j * 128, 128)],
                                         rhs=KT_sb[ds(po, 64), pt, ds(m * 512, wfree)],
                                         start=True, stop=True,
                                         tile_position=(po, 0))
                    bnst = wp.tile([128, 2, 6], F32, tag="bnst")
                    nbc = (Wj + 511) // 512
                    for cch in range(nbc):
                        cw = min(512, Wj - cch * 512)
                        nc.vector.bn_stats(out=bnst[:, cch, :],
                                           in_=ps[:, ds(cch * 512, cw)])
                    nc.vector.bn_aggr(out=mv[:, h, :], in_=bnst[:, :nbc, :])
                    if layer == 0:
                        nc.vector.tensor_tensor(out=att[:, h, :Wj], in0=ps[:, :Wj],
                                                in1=msk[:, :Wj], op=ALU.add)
                    else:
                        nc.scalar.activation(out=att[:, h, :Wj], in_=ps[:, :Wj],
                                             func=AF.Copy, scale=1.0)

                # ---- selection init ----
                nc.scalar.activation(out=sd[:], in_=mv[:, :, 1], func=AF.Sqrt, scale=1.0)
                nc.vector.tensor_scalar(out=sdc0[:], in0=sd[:], scalar1=selt(layer, j, 1),
                                        scalar2=None, op0=ALU.mult)
                nc.vector.tensor_scalar(out=t_[:], in0=sd[:], scalar1=selt(layer, j, 0),
                                        scalar2=None, op0=ALU.mult)
                nc.vector.tensor_tensor(out=t_[:], in0=t_[:], in1=mv[:, :, 0], op=ALU.add)
                nc.vector.tensor_scalar(out=lo[:], in0=sd[:], scalar1=-8.0, scalar2=None,
                                        op0=ALU.mult)
                nc.vector.tensor_tensor(out=lo[:], in0=lo[:], in1=mv[:, :, 0], op=ALU.add)
                nc.vector.tensor_scalar(out=hi[:], in0=sd[:], scalar1=8.0, scalar2=None,
                                        op0=ALU.mult)
                nc.vector.tensor_tensor(out=hi[:], in0=hi[:], in1=mv[:, :, 0], op=ALU.add)
                nc.vector.tensor_scalar(out=SL[:, 0, :], in0=ones8[:],
                                        scalar1=selt(layer, j, 2), scalar2=None, op0=ALU.mult)
                nc.vector.memset(SL[:, 1, :], 1.0)
                nc.vector.tensor_scalar(out=SH[:, 0, :], in0=ones8[:],
                                        scalar1=selt(layer, j, 3), scalar2=-1.0,
                                        op0=ALU.mult, op1=ALU.mult)
                nc.vector.memset(SH[:, 1, :], 1.0)
                nc.vector.memset(newv[:, 1, :], 1.0)

                # ---- iterations ----
                for it in range(B_NEWTON + B_ILL):
                    for h in range(H):
                        junk = scp.tile([128, S], F32, tag="scr1024", name="junk")
                        nc.vector.scalar_tensor_tensor(out=junk[:, :Wj], in0=att[:, h, :Wj],
                                                       scalar=t_[:, h:h + 1],
                                                       in1=ones1[:].to_broadcast([128, Wj]),
                                                       op0=ALU.is_ge, op1=ALU.mult,
                                                       accum_out=cnt[:, h:h + 1])
                    nc.vector.tensor_scalar(out=f, in0=cnt[:], scalar1=selt(layer, j, 3),
                                            scalar2=None, op0=ALU.subtract)
                    nc.vector.tensor_scalar(out=ge[:], in0=f, scalar1=0.0, scalar2=None,
                                            op0=ALU.is_ge)
                    nc.vector.tensor_scalar(out=nge[:], in0=f, scalar1=0.0, scalar2=None,
                                            op0=ALU.is_lt)
                    nc.vector.tensor_scalar(out=SL[:, 1, :], in0=SL[:, 1, :], scalar1=0.5,
                                            scalar2=None, op0=ALU.mult)
                    nc.vector.tensor_scalar(out=SH[:, 1, :], in0=SH[:, 1, :], scalar1=0.5,
                                            scalar2=None, op0=ALU.mult)
                    nc.vector.copy_predicated(lo[:], ge[:], t_[:])
                    nc.vector.copy_predicated(hi[:], nge[:], t_[:])
                    nc.vector.copy_predicated(
                        SL[:, :, :], ge[:, None, :].to_broadcast([128, 2, 8]), newv[:, :, :])
                    nc.vector.copy_predicated(
                        SH[:, :, :], nge[:, None, :].to_broadcast([128, 2, 8]), newv[:, :, :])
                    if it < B_NEWTON:
                        tgt = [0.0, SLACK, -SLACK][it]
                        nc.vector.tensor_scalar(out=stp[:], in0=f, scalar1=tgt, scalar2=None,
                                                op0=ALU.subtract)
                        nc.vector.tensor_tensor(out=stp[:], in0=stp[:], in1=sdc0[:],
                                                op=ALU.mult)
                        nc.vector.tensor_tensor(out=t_[:], in0=t_[:], in1=stp[:], op=ALU.add)
                        nc.vector.tensor_tensor(out=t_[:], in0=t_[:], in1=lo[:], op=ALU.max)
                        nc.vector.tensor_tensor(out=t_[:], in0=t_[:], in1=hi[:], op=ALU.min)
                    else:
                        fl = sp.tile([128, 8], F32, tag="fl")
                        fh = sp.tile([128, 8], F32, tag="fh")
                        den = sp.tile([128, 8], F32, tag="den")
                        num = sp.tile([128, 8], F32, tag="num")
                        nc.vector.tensor_tensor(out=fl[:], in0=SL[:, 0, :], in1=SL[:, 1, :],
                                                op=ALU.mult)
                        nc.vector.tensor_tensor(out=fh[:], in0=SH[:, 0, :], in1=SH[:, 1, :],
                                                op=ALU.mult)
                        nc.vector.tensor_tensor(out=den[:], in0=fh[:], in1=fl[:],
                                                op=ALU.subtract)
                        nc.vector.reciprocal(out=den[:], in_=den[:])
                        nc.vector.tensor_tensor(out=num[:], in0=lo[:], in1=fh[:], op=ALU.mult)
                        nc.vector.tensor_tensor(out=stp[:], in0=hi[:], in1=fl[:], op=ALU.mult)
                        nc.vector.tensor_tensor(out=num[:], in0=num[:], in1=stp[:],
                                                op=ALU.subtract)
                        nc.vector.tensor_tensor(out=t_[:], in0=num[:], in1=den[:],
                                                op=ALU.mult)

                # ---- finisher ----
                idx = sp.tile([128, 8], F32, tag="idx")
                nc.vector.tensor_scalar(out=idx[:], in0=SL[:, 0, :], scalar1=0.5,
                                        scalar2=0.0, op0=ALU.subtract, op1=ALU.max)
                nc.vector.tensor_scalar(out=idx[:], in0=idx[:], scalar1=7.0, scalar2=None,
                                        op0=ALU.min)
                tstar = sp.tile([128, 8], F32, tag="tstar")
                for h in range(H):
                    wd = scp.tile([128, S], F32, tag="scr1024", name="wd")
                    nc.vector.scalar_tensor_tensor(out=wd[:, :Wj], in0=att[:, h, :Wj],
                                                   scalar=lo[:, h:h + 1],
                                                   in1=drop1[:].to_broadcast([128, Wj]),
                                                   op0=ALU.is_lt, op1=ALU.mult)
                    u = scp.tile([128, S], F32, tag="scr1024", name="u")
                    nc.vector.scalar_tensor_tensor(out=u[:, :Wj], in0=att[:, h, :Wj],
                                                   scalar=-1.0, in1=wd[:, :Wj],
                                                   op0=ALU.mult, op1=ALU.add)
                    u8 = wp.tile([128, 8], F32, tag="u8")
                    nc.vector.max(out=u8[:], in_=u[:, :Wj])
                    sel8 = wp.tile([128, 8], F32, tag="sel8")
                    nc.vector.tensor_tensor(out=sel8[:], in0=iota_sb[:],
                                            in1=idx[:, h:h + 1].to_broadcast([128, 8]),
                                            op=ALU.is_equal)
                    nc.vector.tensor_tensor(out=sel8[:], in0=sel8[:], in1=u8[:], op=ALU.mult)
                    nc.vector.tensor_reduce(tstar[:, h:h + 1], sel8[:], op=ALU.add, axis=AXX)
                nc.vector.tensor_scalar(out=tstar[:], in0=tstar[:], scalar1=-1.0,
                                        scalar2=None, op0=ALU.mult)

                # ---- exp, mask+Z, transpose, ctx ----
                psA = pc.tile([128, 512], F32, tag="psA")
                psB = pc.tile([128, 512], F32, tag="psB")
                for h in range(H):
                    e = scp.tile([128, S], F32, tag="scr1024", name="esb")
                    nc.scalar.activation(out=e[:, :Wj], in_=att[:, h, :Wj], func=AF.Exp,
                                         bias=cNEB[:], scale=1.0 / SCALE)
                    me = wp.tile([128, S], BF16, tag="mebf")
                    nc.vector.scalar_tensor_tensor(out=me[:, :Wj], in0=att[:, h, :Wj],
                                                   scalar=tstar[:, h:h + 1], in1=e[:, :Wj],
                                                   op0=ALU.is_ge, op1=ALU.mult,
                                                   accum_out=zrec[:, h:h + 1])
                    eT = wp.tile([128, 8, 128], BF16, tag="eT")
                    for kt in range(nkt):
                        nc.sync.dma_start_transpose(eT[:, kt, :], me[:, ts(kt, 128)])
                    tt, po = h // 2, (h % 2) * 64
                    for kt in range(nkt):
                        nc.tensor.matmul(psA[ds(po, 64), ts(tt, 128)],
                                         lhsT=V_sb[:, kt, ds(h * 64, 64)],
                                         rhs=eT[:, kt, :],
                                         start=(kt == 0), stop=(kt == nkt - 1),
                                         tile_position=(0, po))
                nc.vector.reciprocal(out=zrec[:], in_=zrec[:])
                for tt in range(4):
                    for kt in range(8):
                        nc.tensor.matmul(psB[:, ts(tt, 128)],
                                         lhsT=V_sb[:, kt, ts(tt, 128)],
                                         rhs=gT_all[:, j, kt, :],
                                         start=(kt == 0), stop=(kt == 7))
                zps = pp.tile([128, 512], F32, tag="ps512")
                nc.tensor.transpose(zps[:8, :128], zrec[:], ident_sb[:])
                zT_s = wp.tile([8, 128], F32, tag="zTs")
                nc.scalar.activation(out=zT_s[:], in_=zps[:8, :128], func=AF.Copy, scale=1.0)
                ctxT = wp.tile([128, 4, 128], BF16, tag="ctxT")
                for tt in range(4):
                    smat = pp.tile([128, 512], F32, tag="ps512")
                    nc.tensor.matmul(smat[:, :128], lhsT=rsel_sb[:, ts(tt, 128)], rhs=zT_s[:],
                                     start=True, stop=True)
                    smat_sb = wp.tile([128, 128], F32, tag="smatsb")
                    nc.scalar.activation(out=smat_sb[:], in_=smat[:, :128], func=AF.Copy,
                                         scale=1.0)
                    tmp = wp.tile([128, 128], F32, tag="ctmp")
                    nc.vector.tensor_tensor(out=tmp[:], in0=psA[:, ts(tt, 128)],
                                            in1=smat_sb[:], op=ALU.mult)
                    nc.vector.tensor_tensor(out=ctxT[:, tt, :], in0=tmp[:],
                                            in1=psB[:, ts(tt, 128)], op=ALU.add)
                hps = pp.tile([128, 512], F32, tag="ps512")
                for tt in range(4):
                    nc.tensor.matmul(hps[:], lhsT=ctxT[:, tt, :], rhs=wo_sb[:, tt, :],
                                     start=(tt == 0), stop=(tt == 3))
                pre = wp.tile([128, D], F32, tag="lnpre")
                nc.vector.tensor_tensor(out=pre[:], in0=hps[:], in1=resid_fn(j), op=ALU.add)
                _layernorm(pre, h_out[:, ds(j * D, D)])

        # ===== layer 1 =====
        gT_dec = cp.tile([128, NSLOT, 8, 128], BF16, tag="gT")
        for j in range(NSLOT):
            softmax_half_T(gdec[j, :, :], gT_dec[:, j, :, :])
        KT_dec = cp.tile([128, 4, S], F32, tag="KTd")
        project_T(yT_sb, wkd_sb, bkd_sb, KT_dec, S)
        QT_dec = cp.tile([128, 4, 512], F32, tag="QTd")
        project_T(yTq_sb, wkd_sb, bkd_sb, QT_dec, 512)
        V_dec = cp.tile([128, 8, 512], BF16, tag="Vd")
        project_V(yT_sb, wvd_sb, V_dec)
        h_l1 = cp.tile([128, NSLOT * D], F32, tag="h_l1")

        def resid_dec(j):
            return yeff_sb[:, ds(j * D, D)]

        attention_layer(0, KT_dec, V_dec, QT_dec, gT_dec, h_l1, resid_dec, wod_sb)

        # ===== layer 2 =====
        wke_sb = load(wk_enc[:, :, :], [128, 4, 512], name="wkd", funnel=True)
        wqe_sb = load(wq_enc[:, :, :], [128, 4, 512], name="yTq", funnel=True)
        wve_sb = load(wv_enc[:, :, :], [128, 4, 512], name="wvd", funnel=True)
        woe_sb = load(wo_enc[:, :, :], [128, 4, 512], BF16, name="wod", funnel=True)
        gT_enc = cp.tile([128, NSLOT, 8, 128], BF16, tag="gT")
        for j in range(NSLOT):
            softmax_half_T(genc[j, :, :], gT_enc[:, j, :, :])
        zT_sb = load(zT[:, :, :], [128, 4, S], name="yT")
        KT_enc = cp.tile([128, 4, S], F32, tag="KTd")
        project_T(zT_sb, wke_sb, bke_sb, KT_enc, S)
        V_enc = cp.tile([128, 8, 512], BF16, tag="Vd")
        project_V(zT_sb, wve_sb, V_enc)
        hT = cp.tile([128, 4, 512], F32, tag="hT")
        for j in range(NSLOT):
            for dt_ in range(4):
                ps = ps512()
                nc.tensor.transpose(ps[:, :128], h_l1[:, ds(j * D + dt_ * 128, 128)],
                                    ident_sb[:])
                nc.scalar.activation(out=hT[:, dt_, ds(j * 128, 128)], in_=ps[:, :128],
                                     func=AF.Copy, scale=1.0)
        QT_enc = cp.tile([128, 4, 512], F32, tag="QTd")
        project_T(hT, wqe_sb, bqe_sb, QT_enc, 512)
        h_l2 = cp.tile([128, NSLOT * D], F32, tag="h_l2")

        def resid_enc(j):
            r = wp.tile([128, D], F32, tag="rese")
            nc.vector.tensor_tensor(out=r[:], in0=h_l1[:, ds(j * D, D)], in1=boe_sb[:],
                                    op=ALU.add)
            return r[:]

        attention_layer(1, KT_enc, V_enc, QT_enc, gT_enc, h_l2, resid_enc, woe_sb)

        # ===== MLP =====
        w1_sb = sp.tile([128, 4, FC], BF16, tag="att", name="w1_sb")
        nc.gpsimd.dma_start(w1_sb[:], w1[:, :, :])
        nc.scalar.copy(w1_sb[:], w1_sb[:])
        w2_sb = cp.tile([128, 16, 512], BF16, tag="KTd", name="w2_sb")
        nc.gpsimd.dma_start(w2_sb[:], w2[:, :, :])
        nc.scalar.copy(w2_sb[:], w2_sb[:])
        for j in range(NSLOT):
            h2T = sp.tile([128, 4, 128], BF16, tag="h2T")
            for dt_ in range(4):
                ps = ps512()
                nc.tensor.transpose(ps[:, :128], h_l2[:, ds(j * D + dt_ * 128, 128)],
                                    ident_sb[:])
                nc.scalar.activation(out=h2T[:, dt_, :], in_=ps[:, :128], func=AF.Copy,
                                     scale=1.0)
            m1T = sp.tile([128, 16, 128], BF16, tag="cmaskt", name="m1T")
            for ft in range(16):
                ps = ps512()
                for dt_ in range(4):
                    nc.tensor.matmul(ps[:, :128], lhsT=w1_sb[:, dt_, ts(ft, 128)],
                                     rhs=h2T[:, dt_, :],
                                     start=(dt_ == 0), stop=(dt_ == 3))
                nc.scalar.activation(out=m1T[:, ft, :], in_=ps[:, :128], func=AF.Relu,
                                     bias=b1_sb[:, ft:ft + 1], scale=1.0)
            h3ps = pp.tile([128, 512], F32, tag="ps512")
            for ft in range(16):
                nc.tensor.matmul(h3ps[:], lhsT=m1T[:, ft, :], rhs=w2_sb[:, ft, :],
                                 start=(ft == 0), stop=(ft == 15))
            pre = wp.tile([128, D], F32, tag="mlppre")
            nc.vector.tensor_tensor(out=pre[:], in0=h3ps[:], in1=h_l2[:, ds(j * D, D)],
                                    op=ALU.add)
            nc.vector.tensor_tensor(out=pre[:], in0=pre[:], in1=b2_sb[:], op=ALU.add)
            o = wp.tile([128, D], F32, tag="osb")
            _layernorm(pre, o[:])
            nc.sync.dma_start(out[j, :, :], o[:])

    from concourse import bacc as _bacc
    _bacc._bass_rust.move_matmul_waits_to_ldweights(nc.m)
    _bacc._bass_rust.generate_event_semaphores(nc)
    return nc


# ---------------------------------------------------------------------------
# Host side
# ---------------------------------------------------------------------------

def _core_inputs(inputs, core):
    n, p = core // 2, core % 2
    G = [p + 2 * j for j in range(NSLOT)]          # global q-tile indices
    qrows = np.concatenate([np.arange(g * 128, g * 128 + 128) for g in G])

    y = np.asarray(inputs["y"], np.float32)[n]     # [S, D]
    z = np.asarray(inputs["z"], np.float32)[n]
    f32 = np.float32

    def P3(arr, a):
        arr = np.asarray(arr)
        return np.ascontiguousarray(arr.reshape(a, 128, arr.shape[-1]).transpose(1, 0, 2))

    def hmat(w):   # [H, D, KD] -> [128, 4, H*KD... pre-permuted [D,H*KD]]
        return P3(np.ascontiguousarray(np.moveaxis(np.asarray(w, f32), 0, 1)
                                       .reshape(D, H * KD)), 4)

    dec_bv_flat = np.asarray(inputs["dec_bv"], f32).reshape(H * VD)
    enc_bv_flat = np.asarray(inputs["enc_bv"], f32).reshape(H * VD)
    bo_dec_eff = np.asarray(inputs["dec_bo"], f32) + dec_bv_flat @ np.asarray(
        inputs["dec_Wo"], f32)
    bo_enc_eff = np.asarray(inputs["enc_bo"], f32) + enc_bv_flat @ np.asarray(
        inputs["enc_Wo"], f32)

    cm = np.zeros((NSLOT, 128, S), f32)
    seltabs = np.zeros((2, NSLOT, 128, 4), f32)
    for j, g in enumerate(G):
        qidx = np.arange(g * 128, g * 128 + 128)
        col = np.arange(S)[None, :]
        cm[j] = np.where(col <= qidx[:, None], 0.0, NEGBIG)
        seltabs[0, j] = _sel_tables(qidx + 1.0)
        seltabs[1, j] = _sel_tables(np.full(128, float(S)))

    rsel = np.zeros((8, 512), f32)
    for h in range(8):
        rsel[h, h * 64:(h + 1) * 64] = 0.5

    d = {
        "yT": P3(np.ascontiguousarray(y.T), 4),
        "yTq": P3(np.ascontiguousarray(y.T[:, qrows]), 4),
        "zT": P3(np.ascontiguousarray(z.T), 4),
        "y_eff": np.ascontiguousarray(np.moveaxis((y[qrows] + bo_dec_eff).reshape(NSLOT, 128, D), 1, 0).reshape(128, NSLOT * D)),
        "gdec": np.ascontiguousarray(np.asarray(inputs["graph_dec"], f32)[qrows]
                                     .reshape(NSLOT, 128, S)),
        "genc": np.ascontiguousarray(np.asarray(inputs["graph_enc"], f32)[qrows]
                                     .reshape(NSLOT, 128, S)),
        "wk_dec": hmat(inputs["dec_Wk"]),
        "wv_dec": hmat(inputs["dec_Wv"]),
        "wk_enc": hmat(inputs["enc_Wk"]),
        "wq_enc": hmat(inputs["enc_Wq"]),
        "wv_enc": hmat(inputs["enc_Wv"]),
        "bk_dec": np.ascontiguousarray(np.asarray(inputs["dec_bk"], f32).T),   # [64, H]
        "bk_enc": np.ascontiguousarray(np.asarray(inputs["enc_bk"], f32).T),
        "bq_enc": np.ascontiguousarray(np.asarray(inputs["enc_bq"], f32).T),
        "wo_dec": P3(np.asarray(inputs["dec_Wo"], f32), 4),
        "wo_enc": P3(np.asarray(inputs["enc_Wo"], f32), 4),
        "bo_enc_b": np.tile(bo_enc_eff[None, :], (128, 1)),
        "w1": P3(np.asarray(inputs["fc_W1"], f32), 4),
        "w2": P3(np.asarray(inputs["fc_W2"], f32), 16),
        "b1T": np.ascontiguousarray(np.asarray(inputs["fc_b1"], f32)
                                    .reshape(FC // 128, 128).T),
        "b2_b": np.tile(np.asarray(inputs["fc_b2"], f32)[None, :], (128, 1)),
        "cmask": cm,
        "seltab": np.ascontiguousarray(np.moveaxis(seltabs, 2, 0).reshape(128, 2 * NSLOT * 4)),
        "rsel": rsel,
        "iota8": np.tile(np.arange(8, dtype=f32)[None, :], (128, 1)),
        "ident": np.eye(128, dtype=f32),
    }
    import ml_dtypes
    for k in ("wo_dec", "wo_enc", "w1", "w2"):
        d[k] = d[k].astype(ml_dtypes.bfloat16)
    return d


_CACHE = {}


def kernel(**inputs):
    from concourse.bass_utils import run_bass_kernel_spmd

    if "nc" not in _CACHE:
        _CACHE["nc"] = build_program()
    nc = _CACHE["nc"]

    core_ids = list(range(8))
    in_maps = [_core_inputs(inputs, c) for c in core_ids]
    res = run_bass_kernel_spmd(nc, in_maps, core_ids)

    out = np.zeros((N, S, D), np.float32)
    for c in core_ids:
        n, p = c // 2, c % 2
        o = res.results[c]["out"]          # [NSLOT, 128, D]
        for j in range(NSLOT):
            g = p + 2 * j
            out[n, g * 128:(g + 1) * 128, :] = o[j]
    return out


# revision 30
# speedup vs baseline: 3138.1261x; 1.0504x over previous
"""Trainium2 Bass kernel for nn_DecoderAttention (sparse kNN attention block).

Sharding: core c handles batch n = c//2, parity p = c%2, owning q-tiles
{p, p+2, p+4, p+6} of the sequence (parity-interleaved for causal load
balance). No collectives: each core computes its 512 output rows end-to-end.

Top-128-of-row selection: 3 Newton + 4 Illinois count-bisection iterations
(fused is_ge+accum DVE passes) + exact max8 finisher that picks the
(count(lo)-k)-th smallest kept value as the threshold (tie-immune).
"""
import sys, math
from contextlib import ExitStack

sys.path.insert(0, "/opt/trn_rl_repo")

import numpy as np
import concourse.bass as bass
from concourse.bacc import Bacc
import concourse.mybir as mybir
from concourse.tile import TileContext
from concourse.bass import ts, ds

F32 = mybir.dt.float32
F32R = mybir.dt.float32r
BF16 = mybir.dt.bfloat16
AF = mybir.ActivationFunctionType
ALU = mybir.AluOpType
AXX = mybir.AxisListType.X

H, KD, VD, KNN = 8, 64, 64, 128
D, FC, N, S = 512, 2048, 4, 1024
SCALE = 8.0
EPS = 1e-5
EB = 4.0            # e = exp(att_raw/(SCALE) - EB)
NEGBIG = -1.0e18    # causal additive mask
DROP = -2.0e21      # finisher drop penalty
B_NEWTON = 3
B_ILL = 4
SLACK = 6.0
NSLOT = 4
W_SLOT = [256, 512, 768, 1024]  # layer-1 active widths per slot (covers both parities)


def _inv_norm(p):
    lo, hi = -8.0, 8.0
    for _ in range(80):
        m = 0.5 * (lo + hi)
        if 0.5 * (1 + math.erf(m / math.sqrt(2))) < p:
            lo = m
        else:
            hi = m
    return 0.5 * (lo + hi)


def _sel_tables(widths):
    w = np.asarray(widths, np.float64)
    k = np.minimum(w, float(KNN))
    pq = np.clip(1.0 - k / w, 1e-4, 1.0 - 1e-6)
    z0 = np.array([_inv_norm(v) for v in pq])
    phi = np.exp(-z0 * z0 / 2) / math.sqrt(2 * math.pi)
    c0 = np.clip(1.0 / (w * phi), 0.0, 1.0)
    flo0 = w - (k - 0.5)
    km = k - 0.5
    return np.stack([z0, c0, flo0, km], -1).astype(np.float32)  # [128, 4]


# ---------------------------------------------------------------------------
def build_program():
    nc = bass.Bass()

    def din(name, shape, dtype=F32):
        return nc.dram_tensor(name, shape, dtype, kind="ExternalInput")

    yT = din("yT", (128, 4, S))
    yTq = din("yTq", (128, 4, 512))
    zT = din("zT", (128, 4, S))
    y_eff = din("y_eff", (128, NSLOT * D))
    gdec = din("gdec", (NSLOT, 128, S))
    genc = din("genc", (NSLOT, 128, S))
    wk_dec = din("wk_dec", (128, 4, 512))
    wv_dec = din("wv_dec", (128, 4, 512))
    wk_enc = din("wk_enc", (128, 4, 512))
    wq_enc = din("wq_enc", (128, 4, 512))
    wv_enc = din("wv_enc", (128, 4, 512))
    bk_dec = din("bk_dec", (64, H))
    bk_enc = din("bk_enc", (64, H))
    bq_enc = din("bq_enc", (64, H))
    wo_dec = din("wo_dec", (128, 4, 512), BF16)
    wo_enc = din("wo_enc", (128, 4, 512), BF16)
    bo_enc_b = din("bo_enc_b", (128, D))
    w1 = din("w1", (128, 4, FC), BF16)
    w2 = din("w2", (128, 16, 512), BF16)
    b1T = din("b1T", (128, FC // 128))
    b2_b = din("b2_b", (128, D))
    cmask = din("cmask", (NSLOT, 128, S))
    seltab = din("seltab", (128, 2 * NSLOT * 4))
    rsel = din("rsel", (8, 512))
    iota8 = din("iota8", (128, 8))
    ident_in = din("ident", (128, 128))
    out = nc.dram_tensor("out", (NSLOT, 128, D), F32, kind="ExternalOutput")

    with TileContext(nc) as tc, ExitStack() as ectx:
        cp = ectx.enter_context(tc.tile_pool(name="const", bufs=1))
        wp = ectx.enter_context(tc.tile_pool(name="work", bufs=2))
        scp = ectx.enter_context(tc.tile_pool(name="scr", bufs=3))
        sp = ectx.enter_context(tc.tile_pool(name="state", bufs=1))
        pp = ectx.enter_context(tc.tile_pool(name="psum", bufs=2, space="PSUM"))
        pc = ectx.enter_context(tc.tile_pool(name="psumctx", bufs=1, space="PSUM"))

        def ps512():
            return pp.tile([128, 512], F32, tag="ps512", name="ps512")

        def load(ap_dram, shape, dtype=F32, pool=cp, name=None, funnel=True):
            t = pool.tile(shape, dtype, tag=name, name=name)
            nc.gpsimd.dma_start(t[:], ap_dram)
            if funnel:
                # collapse the multi-queue DMA into a single-producer so
                # LDWEIGHTS consumers only need one sync wait
                nc.scalar.copy(t[:], t[:])
            return t

        yT_sb = load(yT[:, :, :], [128, 4, S], name="yT")
        yTq_sb = load(yTq[:, :, :], [128, 4, 512], name="yTq")
        
        wkd_sb = load(wk_dec[:, :, :], [128, 4, 512], name="wkd", funnel=True)
        wvd_sb = load(wv_dec[:, :, :], [128, 4, 512], name="wvd", funnel=True)
        
        
        
        bkd_sb = load(bk_dec[:, :], [64, H], name="bkd")
        bke_sb = load(bk_enc[:, :], [64, H], name="bke")
        bqe_sb = load(bq_enc[:, :], [64, H], name="bqe")
        wod_sb = load(wo_dec[:, :, :], [128, 4, 512], BF16, name="wod", funnel=True)
        
        boe_sb = load(bo_enc_b[:, :], [128, D], name="boe")
        b1_sb = load(b1T[:, :], [128, FC // 128], name="b1")
        b2_sb = load(b2_b[:, :], [128, D], name="b2")
        selt_sb = load(seltab[:, :], [128, 2 * NSLOT * 4], name="selt")
        rsel_sb = load(rsel[:, :], [8, 512], name="rsel", funnel=True)
        iota_sb = load(iota8[:, :], [128, 8], name="iota8")
        yeff_sb = load(y_eff[:, :], [128, NSLOT * D], name="yeff")
        ident_sb = load(ident_in[:, :], [128, 128], name="ident", funnel=True)

        ones8 = cp.tile([128, 8], F32, tag="ones8")
        nc.vector.memset(ones8[:], 1.0)
        ones1 = cp.tile([128, 1], F32, tag="ones1")
        nc.vector.memset(ones1[:], 1.0)
        drop1 = cp.tile([128, 1], F32, tag="drop1")
        nc.vector.memset(drop1[:], DROP)
        cNEB = cp.tile([128, 1], F32, tag="cNEB")
        nc.vector.memset(cNEB[:], -EB)
        cEPS = cp.tile([128, 1], F32, tag="cEPS")
        nc.vector.memset(cEPS[:], EPS)

        def selt(layer, slot, col):
            c = ((layer * NSLOT) + slot) * 4 + col
            return selt_sb[:, c:c + 1]

        # ------------------------------------------------------------------
        def softmax_half_T(g_dram_slot, out_gT):
            g = scp.tile([128, S], F32, tag="scr1024", name="g")
            nc.gpsimd.dma_start(g[:], g_dram_slot)
            mx = wp.tile([128, 1], F32, tag="gmx")
            nc.vector.tensor_reduce(mx[:], g[:], op=ALU.max, axis=AXX)
            nmx = wp.tile([128, 1], F32, tag="gnmx")
            nc.vector.tensor_scalar(out=nmx[:], in0=mx[:], scalar1=-1.0, scalar2=None,
                                    op0=ALU.mult)
            e = scp.tile([128, S], F32, tag="scr1024", name="gse")
            ssum = wp.tile([128, 1], F32, tag="gsum")
            nc.scalar.activation(out=e[:], in_=g[:], func=AF.Exp, bias=nmx[:], scale=1.0,
                                 accum_out=ssum[:])
            rec = wp.tile([128, 1], F32, tag="grec")
            nc.vector.reciprocal(out=rec[:], in_=ssum[:])
            nc.vector.tensor_scalar(out=rec[:], in0=rec[:], scalar1=0.5, scalar2=None,
                                    op0=ALU.mult)
            gb = wp.tile([128, S], BF16, tag="gbf")
            nc.vector.tensor_scalar(out=gb[:], in0=e[:], scalar1=rec[:], scalar2=None,
                                    op0=ALU.mult)
            for kt in range(8):
                nc.sync.dma_start_transpose(out_gT[:, kt, :], gb[:, ts(kt, 128)])

        def project_T(xT_sb, w_sb, b_sb, outT, width):
            """outT [128, 4, width] f32, head h at partitions (h%2)*64..+64 of pair h//2."""
            for h in range(H):
                pt, po = h // 2, (h % 2) * 64
                nmm = (width + 511) // 512
                for m in range(nmm):
                    wfree = min(512, width - m * 512)
                    ps = ps512()
                    for dt_ in range(4):
                        nc.tensor.matmul(ps[:64, :wfree],
                                         lhsT=w_sb[:, dt_, ds(h * 64, 64)],
                                         rhs=xT_sb[:, dt_, ds(m * 512, wfree)],
                                         start=(dt_ == 0), stop=(dt_ == 3))
                    nc.scalar.activation(out=outT[ds(po, 64), pt, ds(m * 512, wfree)],
                                         in_=ps[:64, :wfree], func=AF.Identity,
                                         bias=b_sb[:, h:h + 1], scale=1.0)

        def project_V(xT_sb, w_sb, outV):
            """outV [128, 8, 512] bf16 = x @ Wv (no bias), k-tile major."""
            for kt in range(8):
                ps = ps512()
                for dt_ in range(4):
                    nc.tensor.matmul(ps[:], lhsT=xT_sb[:, dt_, ts(kt, 128)],
                                     rhs=w_sb[:, dt_, :],
                                     start=(dt_ == 0), stop=(dt_ == 3))
                nc.scalar.activation(out=outV[:, kt, :], in_=ps[:], func=AF.Copy, scale=1.0)

        def _layernorm(x_sb, out_ap):
            st = wp.tile([128, 1, 6], F32, tag="lnst")
            nc.vector.bn_stats(out=st[:], in_=x_sb[:, :])
            ag = wp.tile([128, 2], F32, tag="lnag")
            nc.vector.bn_aggr(out=ag[:], in_=st[:])
            sdv = wp.tile([128, 1], F32, tag="lnsd")
            nc.scalar.activation(out=sdv[:], in_=ag[:, 1:2], func=AF.Sqrt, bias=cEPS[:], scale=1.0)
            nc.vector.reciprocal(out=sdv[:], in_=sdv[:])
            nc.vector.tensor_scalar(out=out_ap, in0=x_sb[:], scalar1=ag[:, 0:1],
                                    scalar2=sdv[:], op0=ALU.subtract, op1=ALU.mult)

        # ------------------------------------------------------------------
        def attention_layer(layer, KT_sb, V_sb, QT_sb, gT_all, h_out, resid_fn, wo_sb):
            for j in range(NSLOT):
                Wj = W_SLOT[j] if layer == 0 else S
                nkt = Wj // 128
                nch = Wj // 256
                att = sp.tile([128, 8, S], F32, tag="att")
                if layer == 0:
                    msk = sp.tile([128, S], F32, tag="cmaskt")
                    nc.gpsimd.dma_start(msk[:, :Wj], cmask[j, :, :Wj])
                t_ = sp.tile([128, 8], F32, tag="t_")
                lo = sp.tile([128, 8], F32, tag="lo")
                hi = sp.tile([128, 8], F32, tag="hi")
                SL = sp.tile([128, 2, 8], F32, tag="SL")    # [FLO, WLO]
                SH = sp.tile([128, 2, 8], F32, tag="SH")
                newv = sp.tile([128, 2, 8], F32, tag="newv")  # [f, ones]
                cnt = sp.tile([128, 8], F32, tag="cnt")
                f = newv[:, 0, :]
                sdc0 = sp.tile([128, 8], F32, tag="sdc0")
                mv = sp.tile([128, 8, 2], F32, tag="mv")
                zrec = sp.tile([128, 8], F32, tag="zrec")
                sd = sp.tile([128, 8], F32, tag="sd")
                ge = sp.tile([128, 8], mybir.dt.uint8, tag="ge")
                nge = sp.tile([128, 8], mybir.dt.uint8, tag="nge")
                stp = sp.tile([128, 8], F32, tag="stp")

                # ---- att matmuls + stats + causal mask ----
                for h in range(H):
                    ps = pp.tile([128, S], F32, tag="ps1024")
                    nmm = (Wj + 511) // 512
                    pt, po = h // 2, (h % 2) * 64
                    for m in range(nmm):
                        wfree = min(512, Wj - m * 512)
                        nc.tensor.matmul(ps[:, ds(m * 512, wfree)],
                                         lhsT=QT_sb[ds(po, 64), pt, ds(j * 128, 128)],
                                         rhs=KT_sb[ds(po, 64), pt, ds(m * 512, wfree)],
                                         start=True, stop=True,
                                         tile_position=(po, 0))
                    bnst = wp.tile([128, 2, 6], F32, tag="bnst")
                    nbc = (Wj + 511) // 512
                    for cch in range(nbc):
                        cw = min(512, Wj - cch * 512)
                        nc.vector.bn_stats(out=bnst[:, cch, :],
                                           in_=ps[:, ds(cch * 512, cw)])
                    nc.vector.bn_aggr(out=mv[:, h, :], in_=bnst[:, :nbc, :])
                    if layer == 0:
                        nc.vector.tensor_tensor(out=att[:, h, :Wj], in0=ps[:, :Wj],
                                                in1=msk[:, :Wj], op=ALU.add)
                    else:
                        nc.scalar.activation(out=att[:, h, :Wj], in_=ps[:, :Wj],
                                             func=AF.Copy, scale=1.0)

                # ---- selection init ----
                nc.scalar.activation(out=sd[:], in_=mv[:, :, 1], func=AF.Sqrt, scale=1.0)
                nc.vector.tensor_scalar(out=sdc0[:], in0=sd[:], scalar1=selt(layer, j, 1),
                                        scalar2=None, op0=ALU.mult)
                nc.vector.tensor_scalar(out=t_[:], in0=sd[:], scalar1=selt(layer, j, 0),
                                        scalar2=None, op0=ALU.mult)
                nc.vector.tensor_tensor(out=t_[:], in0=t_[:], in1=mv[:, :, 0], op=ALU.add)
                nc.vector.tensor_scalar(out=lo[:], in0=sd[:], scalar1=-8.0, scalar2=None,
                                        op0=ALU.mult)
                nc.vector.tensor_tensor(out=lo[:], in0=lo[:], in1=mv[:, :, 0], op=ALU.add)
                nc.vector.tensor_scalar(out=hi[:], in0=sd[:], scalar1=8.0, scalar2=None,
                                        op0=ALU.mult)
                nc.vector.tensor_tensor(out=hi[:], in0=hi[:], in1=mv[:, :, 0], op=ALU.add)
                nc.vector.tensor_scalar(out=SL[:, 0, :], in0=ones8[:],
                                        scalar1=selt(layer, j, 2), scalar2=None, op0=ALU.mult)
                nc.vector.memset(SL[:, 1, :], 1.0)
                nc.vector.tensor_scalar(out=SH[:, 0, :], in0=ones8[:],
                                        scalar1=selt(layer, j, 3), scalar2=-1.0,
                                        op0=ALU.mult, op1=ALU.mult)
                nc.vector.memset(SH[:, 1, :], 1.0)
                nc.vector.memset(newv[:, 1, :], 1.0)
                kap = sp.tile([128, 1], F32, tag="kap")
                nc.vector.tensor_scalar(out=kap[:], in0=selt(layer, j, 3), scalar1=2.0,
                                        scalar2=float(Wj), op0=ALU.mult, op1=ALU.subtract)
                tneg = sp.tile([128, 8], F32, tag="tneg")
                sact = sp.tile([128, 8], F32, tag="sact")

                # ---- iterations ----
                for it in range(B_NEWTON + B_ILL):
                    nc.vector.tensor_scalar(out=tneg[:], in0=t_[:], scalar1=-1.0,
                                            scalar2=None, op0=ALU.mult)
                    for h in range(4):
                        junk = scp.tile([128, S], F32, tag="scr1024", name="junk")
                        nc.vector.scalar_tensor_tensor(out=junk[:, :Wj], in0=att[:, h, :Wj],
                                                       scalar=t_[:, h:h + 1],
                                                       in1=ones1[:].to_broadcast([128, Wj]),
                                                       op0=ALU.is_ge, op1=ALU.mult,
                                                       accum_out=cnt[:, h:h + 1])
                    for h in range(4, H):
                        junk2 = scp.tile([128, S], F32, tag="scr1024", name="junk2")
                        nc.scalar.activation(out=junk2[:, :Wj], in_=att[:, h, :Wj],
                                             func=AF.Sign, bias=tneg[:, h:h + 1], scale=1.0,
                                             accum_out=sact[:, h:h + 1])
                    nc.vector.tensor_scalar(out=f[:, 0:4], in0=cnt[:, 0:4],
                                            scalar1=selt(layer, j, 3),
                                            scalar2=None, op0=ALU.subtract)
                    nc.vector.tensor_scalar(out=f[:, 4:8], in0=sact[:, 4:8],
                                            scalar1=kap[:], scalar2=0.5,
                                            op0=ALU.subtract, op1=ALU.mult)
                    nc.vector.tensor_scalar(out=ge[:], in0=f, scalar1=0.0, scalar2=None,
                                            op0=ALU.is_ge)
                    nc.vector.tensor_scalar(out=nge[:], in0=f, scalar1=0.0, scalar2=None,
                                            op0=ALU.is_lt)
                    nc.vector.tensor_scalar(out=SL[:, 1, :], in0=SL[:, 1, :], scalar1=0.5,
                                            scalar2=None, op0=ALU.mult)
                    nc.vector.tensor_scalar(out=SH[:, 1, :], in0=SH[:, 1, :], scalar1=0.5,
                                            scalar2=None, op0=ALU.mult)
                    nc.vector.copy_predicated(lo[:], ge[:], t_[:])
                    nc.vector.copy_predicated(hi[:], nge[:], t_[:])
                    nc.vector.copy_predicated(
                        SL[:, :, :], ge[:, None, :].to_broadcast([128, 2, 8]), newv[:, :, :])
                    nc.vector.copy_predicated(
                        SH[:, :, :], nge[:, None, :].to_broadcast([128, 2, 8]), newv[:, :, :])
                    if it < B_NEWTON:
                        tgt = [0.0, SLACK, -SLACK][it]
                        nc.vector.tensor_scalar(out=stp[:], in0=f, scalar1=tgt, scalar2=None,
                                                op0=ALU.subtract)
                        nc.vector.tensor_tensor(out=stp[:], in0=stp[:], in1=sdc0[:],
                                                op=ALU.mult)
                        nc.vector.tensor_tensor(out=t_[:], in0=t_[:], in1=stp[:], op=ALU.add)
                        nc.vector.tensor_tensor(out=t_[:], in0=t_[:], in1=lo[:], op=ALU.max)
                        nc.vector.tensor_tensor(out=t_[:], in0=t_[:], in1=hi[:], op=ALU.min)
                    else:
                        fl = sp.tile([128, 8], F32, tag="fl")
                        fh = sp.tile([128, 8], F32, tag="fh")
                        den = sp.tile([128, 8], F32, tag="den")
                        num = sp.tile([128, 8], F32, tag="num")
                        nc.vector.tensor_tensor(out=fl[:], in0=SL[:, 0, :], in1=SL[:, 1, :],
                                                op=ALU.mult)
                        nc.vector.tensor_tensor(out=fh[:], in0=SH[:, 0, :], in1=SH[:, 1, :],
                                                op=ALU.mult)
                        nc.vector.tensor_tensor(out=den[:], in0=fh[:], in1=fl[:],
                                                op=ALU.subtract)
                        nc.vector.reciprocal(out=den[:], in_=den[:])
                        nc.vector.tensor_tensor(out=num[:], in0=lo[:], in1=fh[:], op=ALU.mult)
                        nc.vector.tensor_tensor(out=stp[:], in0=hi[:], in1=fl[:], op=ALU.mult)
                        nc.vector.tensor_tensor(out=num[:], in0=num[:], in1=stp[:],
                                                op=ALU.subtract)
                        nc.vector.tensor_tensor(out=t_[:], in0=num[:], in1=den[:],
                                                op=ALU.mult)

                # ---- finisher ----
                idx = sp.tile([128, 8], F32, tag="idx")
                nc.vector.tensor_scalar(out=idx[:], in0=SL[:, 0, :], scalar1=0.5,
                                        scalar2=0.0, op0=ALU.subtract, op1=ALU.max)
                nc.vector.tensor_scalar(out=idx[:], in0=idx[:], scalar1=7.0, scalar2=None,
                                        op0=ALU.min)
                tstar = sp.tile([128, 8], F32, tag="tstar")
                for h in range(H):
                    wd = scp.tile([128, S], F32, tag="scr1024", name="wd")
                    nc.vector.scalar_tensor_tensor(out=wd[:, :Wj], in0=att[:, h, :Wj],
                                                   scalar=lo[:, h:h + 1],
                                                   in1=drop1[:].to_broadcast([128, Wj]),
                                                   op0=ALU.is_lt, op1=ALU.mult)
                    u = scp.tile([128, S], F32, tag="scr1024", name="u")
                    nc.vector.scalar_tensor_tensor(out=u[:, :Wj], in0=att[:, h, :Wj],
                                                   scalar=-1.0, in1=wd[:, :Wj],
                                                   op0=ALU.mult, op1=ALU.add)
                    u8 = wp.tile([128, 8], F32, tag="u8")
                    nc.vector.max(out=u8[:], in_=u[:, :Wj])
                    sel8 = wp.tile([128, 8], F32, tag="sel8")
                    nc.vector.tensor_tensor(out=sel8[:], in0=iota_sb[:],
                                            in1=idx[:, h:h + 1].to_broadcast([128, 8]),
                                            op=ALU.is_equal)
                    nc.vector.tensor_tensor(out=sel8[:], in0=sel8[:], in1=u8[:], op=ALU.mult)
                    nc.vector.tensor_reduce(tstar[:, h:h + 1], sel8[:], op=ALU.add, axis=AXX)
                nc.vector.tensor_scalar(out=tstar[:], in0=tstar[:], scalar1=-1.0,
                                        scalar2=None, op0=ALU.mult)

                # ---- exp, mask+Z, transpose, ctx ----
                psA = pc.tile([128, 512], F32, tag="psA")
                psB = pc.tile([128, 512], F32, tag="psB")
                for h in range(H):
                    e = scp.tile([128, S], F32, tag="scr1024", name="esb")
                    nc.scalar.activation(out=e[:, :Wj], in_=att[:, h, :Wj], func=AF.Exp,
                                         bias=cNEB[:], scale=1.0 / SCALE)
                    me = wp.tile([128, S], BF16, tag="mebf")
                    nc.vector.scalar_tensor_tensor(out=me[:, :Wj], in0=att[:, h, :Wj],
                                                   scalar=tstar[:, h:h + 1], in1=e[:, :Wj],
                                                   op0=ALU.is_ge, op1=ALU.mult,
                                                   accum_out=zrec[:, h:h + 1])
                    eT = wp.tile([128, 8, 128], BF16, tag="eT")
                    for kt in range(nkt):
                        nc.sync.dma_start_transpose(eT[:, kt, :], me[:, ts(kt, 128)])
                    tt, po = h // 2, (h % 2) * 64
                    for kt in range(nkt):
                        nc.tensor.matmul(psA[ds(po, 64), ts(tt, 128)],
                                         lhsT=V_sb[:, kt, ds(h * 64, 64)],
                                         rhs=eT[:, kt, :],
                                         start=(kt == 0), stop=(kt == nkt - 1),
                                         tile_position=(0, po))
                nc.vector.reciprocal(out=zrec[:], in_=zrec[:])
                for tt in range(4):
                    for kt in range(8):
                        nc.tensor.matmul(psB[:, ts(tt, 128)],
                                         lhsT=V_sb[:, kt, ts(tt, 128)],
                                         rhs=gT_all[:, j, kt, :],
                                         start=(kt == 0), stop=(kt == 7))
                zps = pp.tile([128, 512], F32, tag="ps512")
                nc.tensor.transpose(zps[:8, :128], zrec[:], ident_sb[:])
                zT_s = wp.tile([8, 128], F32, tag="zTs")
                nc.scalar.activation(out=zT_s[:], in_=zps[:8, :128], func=AF.Copy, scale=1.0)
                ctxT = wp.tile([128, 4, 128], BF16, tag="ctxT")
                for tt in range(4):
                    smat = pp.tile([128, 512], F32, tag="ps512")
                    nc.tensor.matmul(smat[:, :128], lhsT=rsel_sb[:, ts(tt, 128)], rhs=zT_s[:],
                                     start=True, stop=True)
                    smat_sb = wp.tile([128, 128], F32, tag="smatsb")
                    nc.scalar.activation(out=smat_sb[:], in_=smat[:, :128], func=AF.Copy,
                                         scale=1.0)
                    tmp = wp.tile([128, 128], F32, tag="ctmp")
                    nc.vector.tensor_tensor(out=tmp[:], in0=psA[:, ts(tt, 128)],
                                            in1=smat_sb[:], op=ALU.mult)
                    nc.vector.tensor_tensor(out=ctxT[:, tt, :], in0=tmp[:],
                                            in1=psB[:, ts(tt, 128)], op=ALU.add)
                hps = pp.tile([128, 512], F32, tag="ps512")
                for tt in range(4):
                    nc.tensor.matmul(hps[:], lhsT=ctxT[:, tt, :], rhs=wo_sb[:, tt, :],
                                     start=(tt == 0), stop=(tt == 3))
                pre = wp.tile([128, D], F32, tag="lnpre")
                nc.vector.tensor_tensor(out=pre[:], in0=hps[:], in1=resid_fn(j), op=ALU.add)
                _layernorm(pre, h_out[:, ds(j * D, D)])

        # ===== layer 1 =====
        gT_dec = cp.tile([128, NSLOT, 8, 128], BF16, tag="gT")
        for j in range(NSLOT):
            softmax_half_T(gdec[j, :, :], gT_dec[:, j, :, :])
        KT_dec = cp.tile([128, 4, S], F32, tag="KTd")
        project_T(yT_sb, wkd_sb, bkd_sb, KT_dec, S)
        QT_dec = cp.tile([128, 4, 512], F32, tag="QTd")
        project_T(yTq_sb, wkd_sb, bkd_sb, QT_dec, 512)
        V_dec = cp.tile([128, 8, 512], BF16, tag="Vd")
        project_V(yT_sb, wvd_sb, V_dec)
        h_l1 = cp.tile([128, NSLOT * D], F32, tag="h_l1")

        def resid_dec(j):
            return yeff_sb[:, ds(j * D, D)]

        attention_layer(0, KT_dec, V_dec, QT_dec, gT_dec, h_l1, resid_dec, wod_sb)

        # ===== layer 2 =====
        wke_sb = load(wk_enc[:, :, :], [128, 4, 512], name="wkd", funnel=True)
        wqe_sb = load(wq_enc[:, :, :], [128, 4, 512], name="yTq", funnel=True)
        wve_sb = load(wv_enc[:, :, :], [128, 4, 512], name="wvd", funnel=True)
        woe_sb = load(wo_enc[:, :, :], [128, 4, 512], BF16, name="wod", funnel=True)
        gT_enc = cp.tile([128, NSLOT, 8, 128], BF16, tag="gT")
        for j in range(NSLOT):
            softmax_half_T(genc[j, :, :], gT_enc[:, j, :, :])
        zT_sb = load(zT[:, :, :], [128, 4, S], name="yT")
        KT_enc = cp.tile([128, 4, S], F32, tag="KTd")
        project_T(zT_sb, wke_sb, bke_sb, KT_enc, S)
        V_enc = cp.tile([128, 8, 512], BF16, tag="Vd")
        project_V(zT_sb, wve_sb, V_enc)
        hT = cp.tile([128, 4, 512], F32, tag="hT")
        for j in range(NSLOT):
            for dt_ in range(4):
                ps = ps512()
                nc.tensor.transpose(ps[:, :128], h_l1[:, ds(j * D + dt_ * 128, 128)],
                                    ident_sb[:])
                nc.scalar.activation(out=hT[:, dt_, ds(j * 128, 128)], in_=ps[:, :128],
                                     func=AF.Copy, scale=1.0)
        QT_enc = cp.tile([128, 4, 512], F32, tag="QTd")
        project_T(hT, wqe_sb, bqe_sb, QT_enc, 512)
        h_l2 = cp.tile([128, NSLOT * D], F32, tag="h_l2")

        def resid_enc(j):
            r = wp.tile([128, D], F32, tag="rese")
            nc.vector.tensor_tensor(out=r[:], in0=h_l1[:, ds(j * D, D)], in1=boe_sb[:],
                                    op=ALU.add)
            return r[:]

        attention_layer(1, KT_enc, V_enc, QT_enc, gT_enc, h_l2, resid_enc, woe_sb)

        # ===== MLP =====
        w1_sb = sp.tile([128, 4, FC], BF16, tag="att", name="w1_sb")
        nc.gpsimd.dma_start(w1_sb[:], w1[:, :, :])
        nc.scalar.copy(w1_sb[:], w1_sb[:])
        w2_sb = cp.tile([128, 16, 512], BF16, tag="KTd", name="w2_sb")
        nc.gpsimd.dma_start(w2_sb[:], w2[:, :, :])
        nc.scalar.copy(w2_sb[:], w2_sb[:])
        for j in range(NSLOT):
            h2T = sp.tile([128, 4, 128], BF16, tag="h2T")
            for dt_ in range(4):
                ps = ps512()
                nc.tensor.transpose(ps[:, :128], h_l2[:, ds(j * D + dt_ * 128, 128)],
                                    ident_sb[:])
                nc.scalar.activation(out=h2T[:, dt_, :], in_=ps[:, :128], func=AF.Copy,
                                     scale=1.0)
            m1T = sp.tile([128, 16, 128], BF16, tag="cmaskt", name="m1T")
            for ft in range(16):
                ps = ps512()
                for dt_ in range(4):
                    nc.tensor.matmul(ps[:, :128], lhsT=w1_sb[:, dt_, ts(ft, 128)],
                                     rhs=h2T[:, dt_, :],
                                     start=(dt_ == 0), stop=(dt_ == 3))
                nc.scalar.activation(out=m1T[:, ft, :], in_=ps[:, :128], func=AF.Relu,
                                     bias=b1_sb[:, ft:ft + 1], scale=1.0)
            h3ps = pp.tile([128, 512], F32, tag="ps512")
            for ft in range(16):
                nc.tensor.matmul(h3ps[:], lhsT=m1T[:, ft, :], rhs=w2_sb[:, ft, :],
                                 start=(ft == 0), stop=(ft == 15))
            pre = wp.tile([128, D], F32, tag="mlppre")
            nc.vector.tensor_tensor(out=pre[:], in0=h3ps[:], in1=h_l2[:, ds(j * D, D)],
                                    op=ALU.add)
            nc.vector.tensor_tensor(out=pre[:], in0=pre[:], in1=b2_sb[:], op=ALU.add)
            o = wp.tile([128, D], F32, tag="osb")
            _layernorm(pre, o[:])
            nc.sync.dma_start(out[j, :, :], o[:])

    from concourse import bacc as _bacc
    _bacc._bass_rust.move_matmul_waits_to_ldweights(nc.m)
    _bacc._bass_rust.generate_event_semaphores(nc)
    return nc


# ---------------------------------------------------------------------------
# Host side
# ---------------------------------------------------------------------------

def _core_inputs(inputs, core):
    n, p = core // 2, core % 2
    G = [p + 2 * j for j in range(NSLOT)]          # global q-tile indices
    qrows = np.concatenate([np.arange(g * 128, g * 128 + 128) for g in G])

    y = np.asarray(inputs["y"], np.float32)[n]     # [S, D]
    z = np.asarray(inputs["z"], np.float32)[n]
    f32 = np.float32

    def P3(arr, a):
        arr = np.asarray(arr)
        return np.ascontiguousarray(arr.reshape(a, 128, arr.shape[-1]).transpose(1, 0, 2))

    def hmat(w):   # [H, D, KD] -> [128, 4, H*KD... pre-permuted [D,H*KD]]
        return P3(np.ascontiguousarray(np.moveaxis(np.asarray(w, f32), 0, 1)
                                       .reshape(D, H * KD)), 4)

    dec_bv_flat = np.asarray(inputs["dec_bv"], f32).reshape(H * VD)
    enc_bv_flat = np.asarray(inputs["enc_bv"], f32).reshape(H * VD)
    bo_dec_eff = np.asarray(inputs["dec_bo"], f32) + dec_bv_flat @ np.asarray(
        inputs["dec_Wo"], f32)
    bo_enc_eff = np.asarray(inputs["enc_bo"], f32) + enc_bv_flat @ np.asarray(
        inputs["enc_Wo"], f32)

    cm = np.zeros((NSLOT, 128, S), f32)
    seltabs = np.zeros((2, NSLOT, 128, 4), f32)
    for j, g in enumerate(G):
        qidx = np.arange(g * 128, g * 128 + 128)
        col = np.arange(S)[None, :]
        cm[j] = np.where(col <= qidx[:, None], 0.0, NEGBIG)
        seltabs[0, j] = _sel_tables(qidx + 1.0)
        seltabs[1, j] = _sel_tables(np.full(128, float(S)))

    rsel = np.zeros((8, 512), f32)
    for h in range(8):
        rsel[h, h * 64:(h + 1) * 64] = 0.5

    d = {
        "yT": P3(np.ascontiguousarray(y.T), 4),
        "yTq": P3(np.ascontiguousarray(y.T[:, qrows]), 4),
        "zT": P3(np.ascontiguousarray(z.T), 4),
        "y_eff": np.ascontiguousarray(np.moveaxis((y[qrows] + bo_dec_eff).reshape(NSLOT, 128, D), 1, 0).reshape(128, NSLOT * D)),
        "gdec": np.ascontiguousarray(np.asarray(inputs["graph_dec"], f32)[qrows]
                                     .reshape(NSLOT, 128, S)),
        "genc": np.ascontiguousarray(np.asarray(inputs["graph_enc"], f32)[qrows]
                                     .reshape(NSLOT, 128, S)),
        "wk_dec": hmat(inputs["dec_Wk"]),
        "wv_dec": hmat(inputs["dec_Wv"]),
        "wk_enc": hmat(inputs["enc_Wk"]),
        "wq_enc": hmat(inputs["enc_Wq"]),
        "wv_enc": hmat(inputs["enc_Wv"]),
        "bk_dec": np.ascontiguousarray(np.asarray(inputs["dec_bk"], f32).T),   # [64, H]
        "bk_enc": np.ascontiguousarray(np.asarray(inputs["enc_bk"], f32).T),
        "bq_enc": np.ascontiguousarray(np.asarray(inputs["enc_bq"], f32).T),
        "wo_dec": P3(np.asarray(inputs["dec_Wo"], f32), 4),
        "wo_enc": P3(np.asarray(inputs["enc_Wo"], f32), 4),
        "bo_enc_b": np.tile(bo_enc_eff[None, :], (128, 1)),
        "w1": P3(np.asarray(inputs["fc_W1"], f32), 4),
        "w2": P3(np.asarray(inputs["fc_W2"], f32), 16),
        "b1T": np.ascontiguousarray(np.asarray(inputs["fc_b1"], f32)
                                    .reshape(FC // 128, 128).T),
        "b2_b": np.tile(np.asarray(inputs["fc_b2"], f32)[None, :], (128, 1)),
        "cmask": cm,
        "seltab": np.ascontiguousarray(np.moveaxis(seltabs, 2, 0).reshape(128, 2 * NSLOT * 4)),
        "rsel": rsel,
        "iota8": np.tile(np.arange(8, dtype=f32)[None, :], (128, 1)),
        "ident": np.eye(128, dtype=f32),
    }
    import ml_dtypes
    for k in ("wo_dec", "wo_enc", "w1", "w2"):
        d[k] = d[k].astype(ml_dtypes.bfloat16)
    return d


_CACHE = {}


def kernel(**inputs):
    from concourse.bass_utils import run_bass_kernel_spmd

    if "nc" not in _CACHE:
        _CACHE["nc"] = build_program()
    nc = _CACHE["nc"]

    core_ids = list(range(8))
    in_maps = [_core_inputs(inputs, c) for c in core_ids]
    res = run_bass_kernel_spmd(nc, in_maps, core_ids)

    out = np.zeros((N, S, D), np.float32)
    for c in core_ids:
        n, p = c // 2, c % 2
        o = res.results[c]["out"]          # [NSLOT, 128, D]
        for j in range(NSLOT):
            g = p + 2 * j
            out[n, g * 128:(g + 1) * 128, :] = o[j]
    return out
